# revision 2
# baseline (speedup 1.0000x reference)
"""Trainium2 Bass kernel for nn_Attention_17489106830121 (v2: table interp).

Math: logits are rank-1 (attn[b,h,n,j] = s[b,n,h]*ks[b,j]), so per core the
attention middle collapses to ONE scalar function w = f(s),
  f(s) = sum_j vs_j e^{s ks_j} / sum_j e^{s ks_j},
evaluated at 32768 points. Instead of materializing the 8.4M-element exp
field, evaluate f by piecewise-linear interpolation on a 127-knot warped
grid:
  t = 64 + 63.3*tanh(s/3)            (fixed warp, t in (0.7, 127.3))
  w(t) = c0 + sum_l c_l * relu(t-l)  (c = 2nd differences of table T)
T_l = f(g_l) is built on device from the conv->LN->k/v path (one rank-3
matmul for stable logits, one [128,256] Exp, DVE reductions + division).
The relu basis IS exact lerp; sim rel err 2.2e-3 vs 2e-2 tolerance.

Device pipeline per core (one batch element):
  - x^T arrives fp16 (host pre-transposed); conv reads a strided
    (kh kw ph pw) view of it directly (no reorder copy).
  - s^T = wqs^T @ x^T (4 column-group PE streams), tanh+affine -> t fp16,
    flattened to [1, 32768] rows duplicated at partitions {0,32,64,96}.
  - d = t - l: 64 rank-2 matmuls [2x128]x[2,512] on 4 row strips.
  - relu: PSUM->SBUF fp16, alternating ScalarE activation / DVE max.
  - gather: w = basis^T c via rank-128 matmuls, 4 interleaved accumulation
    groups at PE column positions {0,32,64,96}, 16 chunks each, mostly-zero
    lhsT variants (c placed at one column) -> PSUM [16-row blocks, 512].
  - y = wts^T @ Mmat + bproj, fp16 out (host upcasts).

Engine-queue order matters: PE = [s, conv, d x64, table, gather x64, y];
the first 16 relus are enqueued before the table's DVE/ScalarE math so the
d->relu pipeline never stalls behind the (conv-gated) table build.
"""

import numpy as np

B, N, C, HEADS, SR = 8, 4096, 256, 8, 4
HC = C // HEADS
SCALE = HC ** -0.5
EPS = 1e-5
HS = 64 // SR            # 16
N2 = HS * HS             # 256
L = 128
BW = 3.0
AMP = 63.3
NCH = 64                 # 512-col chunks of the 32768 (h,n) columns

_NC_CACHE = {}


def _build_nc(debug=False):
    import concourse.bass as bass
    import concourse.bacc as bacc
    import concourse.mybir as mybir
    from concourse import tile

    dt = mybir.dt
    f32, f16 = dt.float32, dt.float16
    AF = mybir.ActivationFunctionType
    ALU = mybir.AluOpType
    AX = mybir.AxisListType

    nc = bacc.Bacc(None, target_bir_lowering=False)

    xt_d = nc.dram_tensor("xt", [2 * 128, N], f16, kind="ExternalInput")
    ws_d = nc.dram_tensor("wsr", [SR * SR * C, C], f16, kind="ExternalInput")
    b32_d = nc.dram_tensor("b32", [128, 1800], f32, kind="ExternalInput")
    b16_d = nc.dram_tensor("b16", [128, 400], f16, kind="ExternalInput")
    y_d = nc.dram_tensor("y", [N, C], f16, kind="ExternalOutput")
    if debug:
        dbg_t = nc.dram_tensor("dbg_t", [8, N], f16, kind="ExternalOutput")
        dbg_T = nc.dram_tensor("dbg_T", [128, 4], f32, kind="ExternalOutput")
        dbg_kv = nc.dram_tensor("dbg_kv", [1, 2 * N2], f32, kind="ExternalOutput")
        dbg_w = nc.dram_tensor("dbg_w", [8, N], f16, kind="ExternalOutput")
        dbg_xn = nc.dram_tensor("dbg_xn", [128, 2 * N2], f32, kind="ExternalOutput")
        dbg_bas = nc.dram_tensor("dbg_bas", [128, 1024], f16, kind="ExternalOutput")

    with tile.TileContext(nc) as tc:
        with tc.tile_pool(name="const", bufs=1) as cp:
            b32 = cp.tile([128, 1800], f32)
            xt = cp.tile([128, 2, N], f16)
            wssb = cp.tile([128, 32, C], f16)
            trhs = cp.tile([97, 8 * N], f16)
            onesd = cp.tile([97, 128], f16)
            wtsT = cp.tile([72, N], f16)
            crepv = cp.tile([128, 256], f16)
            mm16 = cp.tile([72, C], f16)
            wqssb = cp.tile([128, 2, HEADS], f16)
            bprep = cp.tile([128, C], f32)
            murep = cp.tile([128, N2], f32)
            rsrep = cp.tile([128, N2], f32)
            ones_row = cp.tile([1, 128], f32)
            ones_col = cp.tile([128, 1], f32)
            mones256 = cp.tile([1, N2], f32)
            eps_sb = cp.tile([128, 1], f32)
            cvo = cp.tile([128, 2, N2], f32)
            xm = cp.tile([128, 2, N2], f32)
            xn = cp.tile([128, 2, N2], f32)
            sq2 = cp.tile([128, 2, N2], f32)
            murow = cp.tile([1, 2, N2], f32)
            varrow = cp.tile([1, N2], f32)
            lvrow = cp.tile([1, N2], f32)
            rstdrow = cp.tile([1, N2], f32)
            ks_r = cp.tile([1, N2], f32)
            vs_r = cp.tile([1, N2], f32)
            kmx = cp.tile([1, 1], f32)
            kmn = cp.tile([1, 1], f32)
            kmsb = cp.tile([1, 2, N2], f32)
            krows = cp.tile([3, N2], f32)
            Esb = cp.tile([128, N2], f32)
            Evm = cp.tile([128, N2], f32)
            numv = cp.tile([128, 1], f32)
            denv = cp.tile([128, 1], f32)
            dinv = cp.tile([128, 1], f32)
            Tcol = cp.tile([128, 1], f32)
            Ts1 = cp.tile([128, 1], f32)
            mcol = cp.tile([128, 1], f32)
            msh = cp.tile([128, 1], f32)
            ccol = cp.tile([128, 1], f32)
            cTrow = cp.tile([1, 128], f32)
            msrow = cp.tile([1, C], f32)
            bprep2 = cp.tile([128, C], f32)

            idsb = b32[:, 0:128]
            miota = b32[:, 1540:1541]
            grid3 = b32[0:3, 128:256]
            E16row = b32[0:1, 256:512]
            wkvsb = b32[:, 512:516].rearrange("p (t h) -> p t h", t=2)
            bsrcol = b32[:, 516:518]
            cbrow = b32[0:1, 518:520]
            bpr_r = b32[0:1, 1284:1540]

            # ----- input DMAs -----
            nc.sync.dma_start(b32[:], b32_d[:])
            xd = xt_d[:].rearrange("(ct p) n -> p ct n", p=128)
            nc.sync.dma_start(xt[:, 0, 0:2048], xd[:, 0, 0:2048])
            nc.scalar.dma_start(xt[:, 0, 2048:4096], xd[:, 0, 2048:4096])
            nc.sync.dma_start(xt[:, 1, 0:2048], xd[:, 1, 0:2048])
            nc.scalar.dma_start(xt[:, 1, 2048:4096], xd[:, 1, 2048:4096])
            nc.gpsimd.dma_start(wssb[:], ws_d[:].rearrange("(t p) c -> p t c", p=128))
            nc.scalar.dma_start(wqssb[:], b16_d[:, 0:16].rearrange("p (t h) -> p t h", t=2))
            nc.sync.dma_start(mm16[0:8, :], b16_d[0:8, 144:400])
            nc.scalar.dma_start(mm16[64:72, :], b16_d[0:8, 144:400])

            nc.vector.memset(ones_row[:], 1.0)
            nc.vector.memset(mones256[:], -1.0)
            nc.vector.memset(eps_sb[:], EPS)
            nc.vector.memset(ones_col[:], 1.0)
            for sp in (0, 32, 64, 96):
                nc.vector.memset(onesd[sp:sp + 1, :], 1.0)

            # ----- replicate bproj row down partitions -----
            with tc.tile_pool(name="psR", bufs=1, space="PSUM") as pR:
                rp = pR.tile([128, C], f32, name="rp", tag="rp")
                nc.tensor.matmul(rp[:], lhsT=ones_row[:], rhs=bpr_r,
                                 start=True, stop=True)
                nc.vector.tensor_copy(bprep[:], rp[:])

            # ----- s^T = wqs^T @ x^T, warp to t, flatten -----
            with tc.tile_pool(name="ssp", bufs=1) as ssp:
                sT16 = ssp.tile([8, N], f16)
                t16 = ssp.tile([8, N], f16)
                with tc.tile_pool(name="psS", bufs=2, space="PSUM") as pS:
                    for k4 in range(2):
                        sps = pS.tile([128, 512], f32, name="sps", tag="sps")
                        for q in range(4):
                            k = 4 * k4 + q
                            cb = 32 * q
                            for ct in range(2):
                                nc.tensor.matmul(
                                    sps[cb:cb + 8, :],
                                    lhsT=wqssb[:, ct, :],
                                    rhs=xt[:, ct, 512 * k:512 * (k + 1)],
                                    start=(ct == 0), stop=(ct == 1),
                                    tile_position=(0, cb),
                                )
                        for q in range(4):
                            k = 4 * k4 + q
                            eng = nc.vector.tensor_copy if k % 2 == 0 else nc.scalar.copy
                            eng(sT16[:, 512 * k:512 * (k + 1)], sps[32 * q:32 * q + 8, :])
                nc.scalar.activation(t16[:], sT16[:], AF.Tanh, scale=1.0 / BW)
                nc.vector.tensor_scalar(t16[:], t16[:], AMP, float(L // 2),
                                        ALU.mult, ALU.add)
                for i, sp in enumerate((0, 32, 64, 96)):
                    eng = (nc.sync, nc.scalar, nc.sync, nc.scalar)[i]
                    eng.dma_start(
                        trhs[sp:sp + 1, :].rearrange("p (h n) -> p h n", h=8),
                        t16[:],
                    )

            # ----- conv (transposed: [c_out, spatial]) + LN stats via PE -----
            xtr = xt[:].rearrange("p ct (ph kh pw kw) -> p ct kh kw ph pw",
                                  ph=16, kh=4, pw=16, kw=4)
            with tc.tile_pool(name="psB", bufs=2, space="PSUM") as pB:
                for mo in range(2):
                    cps = pB.tile([128, N2], f32, name="cps", tag="cps")
                    for ct in range(2):
                        for kh in range(4):
                            for kw in range(4):
                                kidx = kh * 8 + kw * 2 + ct
                                cnt = ct * 16 + kh * 4 + kw
                                nc.tensor.matmul(
                                    cps[:],
                                    lhsT=wssb[:, kidx, 128 * mo:128 * (mo + 1)],
                                    rhs=xtr[:, ct, kh, kw],
                                    start=(cnt == 0), stop=(cnt == 31),
                                )
                    nc.vector.tensor_scalar(cvo[:, mo, :], cps[:],
                                            bsrcol[:, mo:mo + 1], None, ALU.add)
                nc.vector.tensor_tensor(sq2[:], cvo[:], cvo[:], ALU.mult)
                muA = pB.tile([1, N2], f32, name="muA", tag="muA")
                muB = pB.tile([1, N2], f32, name="muB", tag="muB")
                for mo in range(2):
                    nc.tensor.matmul(muA[:], lhsT=ones_col[:],
                                     rhs=cvo[:, mo, :], start=(mo == 0),
                                     stop=(mo == 1))
                    nc.tensor.matmul(muB[:], lhsT=ones_col[:],
                                     rhs=sq2[:, mo, :], start=(mo == 0),
                                     stop=(mo == 1))
                nc.vector.tensor_scalar(murow[:, 0, :], muA[:], 1.0 / N2, None,
                                        ALU.mult)
                nc.vector.tensor_scalar(murow[:, 1, :], muB[:], 1.0 / N2, None,
                                        ALU.mult)
                nc.vector.tensor_tensor(varrow[:], murow[:, 0, :], murow[:, 0, :],
                                        ALU.mult)
                nc.vector.tensor_tensor(varrow[:], murow[:, 1, :], varrow[:],
                                        ALU.subtract)
                nc.scalar.activation(lvrow[:], varrow[:], AF.Ln,
                                     bias=eps_sb[0:1, :])
                nc.scalar.activation(rstdrow[:], lvrow[:], AF.Exp, scale=-0.5)
                mrp = pB.tile([128, 2, N2], f32, name="mrp", tag="mrp")
                nc.tensor.matmul(mrp[:, 0, :], lhsT=ones_row[:], rhs=murow[:, 0, :],
                                 start=True, stop=True)
                nc.tensor.matmul(mrp[:, 1, :], lhsT=ones_row[:], rhs=rstdrow[:],
                                 start=True, stop=True)
                nc.vector.tensor_copy(murep[:], mrp[:, 0, :])
                nc.vector.tensor_copy(rsrep[:], mrp[:, 1, :])
                for mo in range(2):
                    nc.vector.tensor_tensor(xm[:, mo, :], cvo[:, mo, :], murep[:],
                                            ALU.subtract)
                    nc.vector.tensor_tensor(xn[:, mo, :], xm[:, mo, :], rsrep[:],
                                            ALU.mult)

            # ----- middle scope -----
            with (
                tc.tile_pool(name="mid", bufs=1) as mp,
                tc.tile_pool(name="pD", bufs=3, space="PSUM") as pD,
                tc.tile_pool(name="pW", bufs=1, space="PSUM") as pW,
                tc.tile_pool(name="pC", bufs=2, space="PSUM") as pC,
                tc.tile_pool(name="pY", bufs=2, space="PSUM") as pY,
                tc.tile_pool(name="ysq", bufs=4) as ysq,
                tc.tile_pool(name="wgq", bufs=2) as wgq,
            ):
                bas = mp.tile([128, NCH, 512], f16)

                # chunk order: within-group index v major, group a minor
                order = []
                for v in range(16):
                    for a in range(4):
                        nbk = 2 * a + v // 8
                        hh = v % 8
                        order.append((a, v, hh * 8 + nbk))

                # d = t - l on 4 row strips (PE queue: right after conv)
                dp_q = {}
                for i, (a, v, cc) in enumerate(order):
                    sp = 32 * (i % 4)
                    dp = pD.tile([128, 512], f32, name=f"dp{cc}", tag="dp")
                    nc.tensor.matmul(
                        dp[:], lhsT=onesd[sp:sp + 1, :],
                        rhs=trhs[sp:sp + 1, 512 * cc:512 * (cc + 1)],
                        start=True, stop=True, tile_position=(sp, 0),
                    )
                    dp_q[cc] = dp

                def relu(i):
                    a, v, cc = order[i]
                    dp = dp_q.pop(cc)
                    if i % 2 == 0:
                        nc.scalar.activation(bas[:, cc, :], dp[:], AF.Relu,
                                             bias=miota)
                    else:
                        nc.vector.tensor_scalar(bas[:, cc, :], dp[:], miota, 0.0,
                                                ALU.add, ALU.max)

                for i in range(16):
                    relu(i)

                # table build: k/v rows (contraction over channel partitions)
                kps_k = pC.tile([1, N2], f32, name="kps_k", tag="tb")
                kps_v = pC.tile([1, N2], f32, name="kps_v", tag="tb")
                for mo in range(2):
                    nc.tensor.matmul(kps_k[:], lhsT=wkvsb[:, mo, 0:1],
                                     rhs=xn[:, mo, :], start=(mo == 0), stop=(mo == 1))
                    nc.tensor.matmul(kps_v[:], lhsT=wkvsb[:, mo, 1:2],
                                     rhs=xn[:, mo, :], start=(mo == 0), stop=(mo == 1))
                nc.vector.tensor_scalar(ks_r[:], kps_k[:], cbrow[0:1, 0:1], None,
                                        ALU.add)
                nc.vector.tensor_scalar(vs_r[:], kps_v[:], cbrow[0:1, 1:2], None,
                                        ALU.add)
                nc.vector.reduce_max(kmx[:], ks_r[:], axis=AX.X)
                nc.vector.tensor_reduce(kmn[:], ks_r[:], axis=AX.X, op=ALU.min)
                kmp = pC.tile([1, 2, N2], f32, name="kmp", tag="tb")
                nc.tensor.matmul(kmp[:, 0, :], lhsT=kmx[:], rhs=mones256[:],
                                 start=True, stop=True)
                nc.tensor.matmul(kmp[:, 1, :], lhsT=kmn[:], rhs=mones256[:],
                                 start=True, stop=True)
                nc.vector.tensor_scalar(kmsb[:], kmp[:], -1.0, None, ALU.mult)
                nc.vector.tensor_scalar(kmsb[:, 0, :], kmsb[:, 0, :], -1.0, None,
                                        ALU.mult)
                nc.vector.tensor_copy(krows[0:1, :], ks_r[:])
                nc.sync.dma_start(krows[1:3, :], kmsb[0:1, :, :])
                # logits X[l,j] = g_l k_j - relu(g_l) kmax + relu(-g_l) kmin
                Xp = pC.tile([128, N2], f32, name="Xp", tag="tb")
                nc.tensor.matmul(Xp[:], lhsT=grid3, rhs=krows[:], start=True,
                                 stop=True)
                nc.scalar.activation(Esb[:], Xp[:], AF.Exp)
                vrp = pC.tile([128, N2], f32, name="vrp", tag="tb")
                nc.tensor.matmul(vrp[:], lhsT=ones_row[:], rhs=vs_r[:],
                                 start=True, stop=True)
                nc.vector.tensor_tensor(Evm[:], Esb[:], vrp[:], ALU.mult)
                nc.vector.reduce_sum(numv[:], Evm[:], axis=AX.X)
                nc.vector.reduce_sum(denv[:], Esb[:], axis=AX.X)
                nc.vector.reciprocal_approx_fast(dinv[:], denv[:])
                nc.vector.tensor_tensor(Tcol[:], numv[:], dinv[:], ALU.mult)
                # second differences -> c (partition l <-> knot l+1)
                nc.vector.memset(Ts1[:], 0.0)
                nc.vector.memset(mcol[:], 0.0)
                nc.vector.memset(msh[:], 0.0)
                nc.scalar.dma_start(Ts1[0:126, :], Tcol[1:127, :])
                nc.vector.tensor_tensor(mcol[0:126, :], Ts1[0:126, :],
                                        Tcol[0:126, :], ALU.subtract)
                nc.sync.dma_start(msh[1:127, :], mcol[0:126, :])
                nc.vector.tensor_tensor(ccol[:], mcol[:], msh[:], ALU.subtract)
                # fold const T_1 into the y bias: bprep2 = bprep + T_1 * msum
                nc.vector.tensor_scalar(msrow[:], b32[0:1, 1544:1800],
                                        Tcol[0:1, 0:1], None, ALU.mult)
                rp2 = pC.tile([128, C], f32, name="rp2", tag="tb")
                nc.tensor.matmul(rp2[:], lhsT=ones_row[:], rhs=msrow[:],
                                 start=True, stop=True)
                nc.vector.tensor_tensor(bprep2[:], rp2[:], bprep[:], ALU.add)
                ctp = pC.tile([1, 128], f32, name="ctp", tag="tb")
                nc.tensor.transpose(ctp[:], ccol[:, 0:1], idsb[:])
                nc.vector.tensor_copy(cTrow[:], ctp[:])
                cvp = pC.tile([128, 256], f32, name="cvp", tag="tb")
                nc.tensor.matmul(cvp[:], lhsT=cTrow[:], rhs=E16row, start=True,
                                 stop=True)
                nc.vector.tensor_copy(crepv[:], cvp[:])

                for i in range(16, NCH):
                    relu(i)

                # gather: 4 interleaved accumulation groups
                wp = pW.tile([128, 512], f32)
                for i, (a, v, cc) in enumerate(order):
                    nc.tensor.matmul(
                        wp[32 * a:32 * a + 16, :],
                        lhsT=crepv[:, 16 * v:16 * (v + 1)],
                        rhs=bas[:, cc, :],
                        start=(v == 0), stop=(v == 15),
                        tile_position=(0, 32 * a),
                        skip_group_check=True,
                    )
                # evac per group, reshape to wts^T [8h, n]
                for a in range(4):
                    wg = wgq.tile([16, 512], f16, name=f"wg{a}", tag="wg")
                    nc.vector.tensor_copy(wg[:], wp[32 * a:32 * a + 16, :])
                    for half in range(2):
                        ncc = 2 * a + half
                        src = wg[8 * half:8 * half + 8, :]
                        eng = nc.sync if half == 0 else nc.scalar
                        eng.dma_start(wtsT[0:8, 512 * ncc:512 * (ncc + 1)], src)
                        eng2 = nc.gpsimd if half == 0 else nc.sync
                        eng2.dma_start(wtsT[64:72, 512 * ncc:512 * (ncc + 1)], src)

                if debug:
                    nc.sync.dma_start(dbg_t[:], trhs[0:1, :].rearrange("p (h n) -> p h n", h=8))
                    nc.sync.dma_start(dbg_T[:, 0:1], Tcol[:])
                    nc.sync.dma_start(dbg_T[:, 1:2], ccol[:])
                    nc.sync.dma_start(dbg_T[:, 2:3], mcol[:])
                    nc.sync.dma_start(dbg_T[:, 3:4], Ts1[:])
                    nc.sync.dma_start(dbg_kv[:, 0:N2], ks_r[:])
                    nc.sync.dma_start(dbg_kv[:, N2:2*N2], vs_r[:])
                    nc.sync.dma_start(dbg_w[:], wtsT[0:8, :])
                    nc.sync.dma_start(dbg_xn[:, 0:N2], xn[:, 0, :])
                    nc.sync.dma_start(dbg_xn[:, N2:2*N2], xn[:, 1, :])
                    nc.sync.dma_start(dbg_bas[:, 0:512], bas[:, 0, :])
                    nc.sync.dma_start(dbg_bas[:, 512:1024], bas[:, 1, :])
                # y = wts^T.T @ Mmat + bproj
                for nb in range(32):
                    sp = 64 * (nb % 2)
                    yp = pY.tile([128, C], f32, name=f"yp{nb}", tag="yp")
                    nc.tensor.matmul(yp[:], lhsT=wtsT[sp:sp + 8, 128 * nb:128 * (nb + 1)],
                                     rhs=mm16[sp:sp + 8, :], start=True, stop=True,
                                     tile_position=(sp, 0))
                    ysb = ysq.tile([128, C], f16, name=f"ysb{nb}", tag="ysb")
                    nc.vector.tensor_tensor(ysb[:], yp[:], bprep2[:], ALU.add)
                    ydma = (nc.gpsimd.dma_start, nc.sync.dma_start,
                            nc.scalar.dma_start)[nb % 3]
                    ydma(y_d[128 * nb:128 * (nb + 1), :], ysb[:])

    nc.compile()
    return nc


def _host_precompute(Wq, Wkv, Wsr, bsr, gamma, beta, Wproj, bproj, k_learn, v_learn):
    lksum = k_learn.reshape(HEADS, HC).sum(1)
    wqs = (Wq.reshape(C, HEADS, HC).sum(2) * (SCALE * lksum)[None, :]).astype(np.float32)
    wk = Wkv[:, 0::2].sum(1)
    wv = Wkv[:, 1::2].sum(1)
    wkv2 = np.stack([gamma * wk, gamma * wv], 1).astype(np.float32)
    lv = v_learn.reshape(HEADS, HC)
    Mmat = np.zeros((HEADS, C), np.float32)
    for h in range(HEADS):
        Mmat[h] = lv[h] @ Wproj[h::HEADS]
    Wsr_flat = np.ascontiguousarray(
        Wsr.transpose(2, 3, 1, 0).reshape(SR * SR * C, C)
    ).astype(np.float16)

    larr = np.arange(1, L + 1).astype(np.float64)
    larr[L - 1] = float(L - 1)
    g = BW * np.arctanh((larr - L / 2) / AMP)
    b32 = np.zeros((128, 1800), np.float32)
    b32[:, 0:128] = np.eye(128, dtype=np.float32)
    b32[0, 128:256] = g
    b32[1, 128:256] = np.maximum(g, 0)
    b32[2, 128:256] = np.maximum(-g, 0)
    e16 = np.zeros((16, 16), np.float32)
    np.fill_diagonal(e16, 1.0)
    b32[0, 256:512] = e16.reshape(-1)
    b32[:, 512:516] = wkv2.reshape(2, 128, 2).transpose(1, 0, 2).reshape(128, 4)
    b32[:, 516:518] = bsr.reshape(2, 128).T
    b32[0, 518] = float((beta * wk).sum())
    b32[0, 519] = float((beta * wv).sum())
    b32[0, 1284:1540] = bproj
    b32[:, 1540] = -(np.arange(128) + 1.0)
    b32[0, 1544:1800] = Mmat.sum(0)

    b16 = np.zeros((128, 400), np.float16)
    b16[:, 0:16] = wqs.reshape(2, 128, HEADS).transpose(1, 0, 2).reshape(128, 16)
    b16[0:8, 144:400] = Mmat.astype(np.float16)
    return dict(wsr=Wsr_flat, b32=b32, b16=b16)


def kernel(**inputs):
    x = np.asarray(inputs["x"], np.float32)
    weights = _host_precompute(
        *[np.asarray(inputs[k], np.float32) for k in
          ("Wq", "Wkv", "Wsr", "bsr", "gamma", "beta", "Wproj", "bproj",
           "k_learn", "v_learn")]
    )
    if "nc" not in _NC_CACHE:
        _NC_CACHE["nc"] = _build_nc()
    nc = _NC_CACHE["nc"]
    in_maps = [
        {"xt": np.ascontiguousarray(x[i].T).astype(np.float16), **weights}
        for i in range(B)
    ]
    from concourse.bass_utils import run_bass_kernel_spmd

    res = run_bass_kernel_spmd(nc, in_maps, core_ids=list(range(B)))
    y = np.stack([res.results[i]["y"].astype(np.float32) for i in range(B)], 0)
    return y


# revision 3
# speedup vs baseline: 1.0043x; 1.0043x over previous
"""Trainium2 Bass kernel for nn_Attention_17489106830121 (v3: L=64 stacked).

Same math as v2 (rank-1 logits -> per-core scalar function f(s) evaluated
by exact piecewise-linear interpolation on a warped grid), with:
  - L=64 knots (sim rel err 5.5e-3 vs 2e-2 tolerance), t = 32+31.3*tanh(s/3)
  - stacked layout: each [128, 1024] PSUM chunk holds TWO 64-row l-halves
    covering 2048 flat (h,n) columns -> relu pass halves (per-column cost)
  - 16 chunks; d = rank-1 broadcast matmuls (2 per chunk, col positions
    0/64, row strips 0/32/64/96); relu folds the knot offset via per-
    partition bias (ScalarE) / fused add,max (DVE), alternating engines
  - gather: 4 groups = (n-half nh, l-half q) at tile_position (64q, 32a),
    8 accumulating matmuls each (variant-8 lhsT, c placed at col h),
    interleaved with the d stream in the PE queue
  - blobs shrunk to ~22KB (b32a [128,8], b32b [4,640], wqs, mm8)
  - y per group right after its evac DMA
"""

import numpy as np

B, N, C, HEADS, SR = 8, 4096, 256, 8, 4
HC = C // HEADS
SCALE = HC ** -0.5
EPS = 1e-5
HS = 64 // SR
N2 = HS * HS             # 256
L = 64
BW = 3.0
AMP = L / 2 - 0.7        # 31.3
NCH = 16                 # [128,1024] stacked chunks; 2048 flat cols each

_NC_CACHE = {}


def _build_nc(debug=False):
    import concourse.bass as bass
    import concourse.bacc as bacc
    import concourse.mybir as mybir
    from concourse import tile

    dt = mybir.dt
    f32, f16 = dt.float32, dt.float16
    AF = mybir.ActivationFunctionType
    ALU = mybir.AluOpType
    AX = mybir.AxisListType

    nc = bacc.Bacc(None, target_bir_lowering=False)

    xt_d = nc.dram_tensor("xt", [2 * 128, N], f16, kind="ExternalInput")
    ws_d = nc.dram_tensor("wsr", [SR * SR * C, C], f16, kind="ExternalInput")
    b32a_d = nc.dram_tensor("b32a", [128, 8], f32, kind="ExternalInput")
    b32b_d = nc.dram_tensor("b32b", [4, 648], f32, kind="ExternalInput")
    wq_d = nc.dram_tensor("wq16", [128, 16], f16, kind="ExternalInput")
    mm_d = nc.dram_tensor("mm8", [8, C], f16, kind="ExternalInput")
    y_d = nc.dram_tensor("y", [N, C], f16, kind="ExternalOutput")
    if debug:
        dbg_t = nc.dram_tensor("dbg_t", [8, N], f16, kind="ExternalOutput")
        dbg_T = nc.dram_tensor("dbg_T", [64, 4], f32, kind="ExternalOutput")
        dbg_kv = nc.dram_tensor("dbg_kv", [1, 2 * N2], f32, kind="ExternalOutput")
        dbg_w = nc.dram_tensor("dbg_w", [8, N], f16, kind="ExternalOutput")
        dbg_xn = nc.dram_tensor("dbg_xn", [128, 2 * N2], f32, kind="ExternalOutput")

    with tile.TileContext(nc) as tc:
        with tc.tile_pool(name="const", bufs=1) as cp:
            b32a = cp.tile([128, 8], f32)
            b32b = cp.tile([4, 648], f32)
            xt = cp.tile([128, 2, N], f16)
            wssb = cp.tile([128, 32, C], f16)
            trhs = cp.tile([97, 8 * N], f16)
            onesd = cp.tile([97, 128], f16)
            wtsT = cp.tile([72, N], f16)
            crepv = cp.tile([128, 64], f16)
            mm16 = cp.tile([72, C], f16)
            wqssb = cp.tile([128, 2, HEADS], f16)
            bprep = cp.tile([128, C], f32)
            murep = cp.tile([128, N2], f32)
            rsrep = cp.tile([128, N2], f32)
            ones_row = cp.tile([1, 128], f32)
            ones_col = cp.tile([128, 1], f32)
            mones256 = cp.tile([1, N2], f32)
            eps_sb = cp.tile([128, 1], f32)
            cvo = cp.tile([128, 2, N2], f32)
            xm = cp.tile([128, 2, N2], f32)
            xn = cp.tile([128, 2, N2], f32)
            sq2 = cp.tile([128, 2, N2], f32)
            murow = cp.tile([1, 2, N2], f32)
            varrow = cp.tile([1, N2], f32)
            lvrow = cp.tile([1, N2], f32)
            rstdrow = cp.tile([1, N2], f32)
            ks_r = cp.tile([1, N2], f32)
            vs_r = cp.tile([1, N2], f32)
            kmx = cp.tile([1, 1], f32)
            kmn = cp.tile([1, 1], f32)
            kmsb = cp.tile([1, 2, N2], f32)
            krows = cp.tile([3, N2], f32)
            Esb = cp.tile([64, N2], f32)
            Evm = cp.tile([64, N2], f32)
            numv = cp.tile([64, 1], f32)
            denv = cp.tile([64, 1], f32)
            dinv = cp.tile([64, 1], f32)
            Tcol = cp.tile([64, 1], f32)
            Ts1 = cp.tile([64, 1], f32)
            mcol = cp.tile([64, 1], f32)
            msh = cp.tile([64, 1], f32)
            ccol = cp.tile([64, 1], f32)
            cTrow = cp.tile([1, 64], f32)
            msrow = cp.tile([1, C], f32)
            bprep2 = cp.tile([128, C], f32)

            miota = b32a[:, 0:1]
            bsrcol = b32a[:, 1:3]
            wkvsb = b32a[:, 3:7].rearrange("p (t h) -> p t h", t=2)
            grid3 = b32b[0:3, 0:64]
            E8row = b32b[0:1, 64:128]
            msum_r = b32b[0:1, 128:384]
            bpr_r = b32b[0:1, 384:640]
            cb_r = b32b[0:1, 640:642]

            # ----- input DMAs: blobs, then xt (4 chunks), then wssb (2) ----
            nc.sync.dma_start(b32a[:], b32a_d[:])
            nc.sync.dma_start(b32b[:], b32b_d[:])
            nc.sync.dma_start(wqssb[:], wq_d[:].rearrange("p (t h) -> p t h", t=2))
            nc.scalar.dma_start(mm16[0:8, :], mm_d[:])
            nc.scalar.dma_start(mm16[64:72, :], mm_d[:])
            xd = xt_d[:].rearrange("(ct p) n -> p ct n", p=128)
            nc.sync.dma_start(xt[:, 0, 0:2048], xd[:, 0, 0:2048])
            nc.scalar.dma_start(xt[:, 1, 0:2048], xd[:, 1, 0:2048])
            nc.sync.dma_start(xt[:, 0, 2048:4096], xd[:, 0, 2048:4096])
            nc.scalar.dma_start(xt[:, 1, 2048:4096], xd[:, 1, 2048:4096])
            nc.gpsimd.dma_start(
                wssb[:, 0:16, :],
                ws_d[0:2048, :].rearrange("(t p) c -> p t c", p=128))
            nc.gpsimd.dma_start(
                wssb[:, 16:32, :],
                ws_d[2048:4096, :].rearrange("(t p) c -> p t c", p=128))

            nc.vector.memset(ones_row[:], 1.0)
            nc.vector.memset(mones256[:], -1.0)
            nc.vector.memset(eps_sb[:], EPS)
            nc.vector.memset(ones_col[:], 1.0)
            for sp in (0, 32, 64, 96):
                nc.vector.memset(onesd[sp:sp + 1, :], 1.0)

            with tc.tile_pool(name="psR", bufs=1, space="PSUM") as pR:
                rp = pR.tile([128, C], f32, name="rp", tag="rp")
                nc.tensor.matmul(rp[:], lhsT=ones_row[:], rhs=bpr_r,
                                 start=True, stop=True)
                nc.vector.tensor_copy(bprep[:], rp[:])

            # ----- s^T, warp to t, flatten to 4 strips -----
            with tc.tile_pool(name="ssp", bufs=1) as ssp:
                sT16 = ssp.tile([8, N], f16)
                t16 = ssp.tile([8, N], f16)
                with tc.tile_pool(name="psS", bufs=2, space="PSUM") as pS:
                    for k4 in range(2):
                        sps = pS.tile([128, 512], f32, name="sps", tag="sps")
                        for q in range(4):
                            k = 4 * k4 + q
                            cb = 32 * q
                            for ct in range(2):
                                nc.tensor.matmul(
                                    sps[cb:cb + 8, :],
                                    lhsT=wqssb[:, ct, :],
                                    rhs=xt[:, ct, 512 * k:512 * (k + 1)],
                                    start=(ct == 0), stop=(ct == 1),
                                    tile_position=(0, cb),
                                )
                        for q in range(4):
                            k = 4 * k4 + q
                            eng = nc.vector.tensor_copy if k % 2 == 0 else nc.scalar.copy
                            eng(sT16[:, 512 * k:512 * (k + 1)], sps[32 * q:32 * q + 8, :])
                nc.scalar.activation(t16[:], sT16[:], AF.Tanh, scale=1.0 / BW)
                nc.vector.tensor_scalar(t16[:], t16[:], AMP, float(L // 2),
                                        ALU.mult, ALU.add)
                for i, sp in enumerate((0, 32, 64, 96)):
                    eng = (nc.sync, nc.scalar, nc.sync, nc.scalar)[i]
                    eng.dma_start(
                        trhs[sp:sp + 1, :].rearrange("p (h n) -> p h n", h=8),
                        t16[:],
                    )

            # ----- conv (out [c_out, spatial]) + LN stats -----
            xtr = xt[:].rearrange("p ct (ph kh pw kw) -> p ct kh kw ph pw",
                                  ph=16, kh=4, pw=16, kw=4)
            with tc.tile_pool(name="psB", bufs=2, space="PSUM") as pB:
                for mo in range(2):
                    cps = pB.tile([128, N2], f32, name="cps", tag="cps")
                    for kh in range(4):
                        for kw in range(4):
                            for ct in range(2):
                                kidx = kh * 8 + kw * 2 + ct
                                cnt = kh * 8 + kw * 2 + ct
                                nc.tensor.matmul(
                                    cps[:],
                                    lhsT=wssb[:, kidx, 128 * mo:128 * (mo + 1)],
                                    rhs=xtr[:, ct, kh, kw],
                                    start=(cnt == 0), stop=(cnt == 31),
                                )
                    nc.vector.tensor_scalar(cvo[:, mo, :], cps[:],
                                            bsrcol[:, mo:mo + 1], None, ALU.add)
                nc.vector.tensor_tensor(sq2[:], cvo[:], cvo[:], ALU.mult)
                muA = pB.tile([1, N2], f32, name="muA", tag="muA")
                muB = pB.tile([1, N2], f32, name="muB", tag="muB")
                for mo in range(2):
                    nc.tensor.matmul(muA[:], lhsT=ones_col[:],
                                     rhs=cvo[:, mo, :], start=(mo == 0),
                                     stop=(mo == 1))
                    nc.tensor.matmul(muB[:], lhsT=ones_col[:],
                                     rhs=sq2[:, mo, :], start=(mo == 0),
                                     stop=(mo == 1))
                nc.vector.tensor_scalar(murow[:, 0, :], muA[:], 1.0 / N2, None,
                                        ALU.mult)
                nc.vector.tensor_scalar(murow[:, 1, :], muB[:], 1.0 / N2, None,
                                        ALU.mult)
                nc.vector.tensor_tensor(varrow[:], murow[:, 0, :], murow[:, 0, :],
                                        ALU.mult)
                nc.vector.tensor_tensor(varrow[:], murow[:, 1, :], varrow[:],
                                        ALU.subtract)
                nc.scalar.activation(lvrow[:], varrow[:], AF.Ln,
                                     bias=eps_sb[0:1, :])
                nc.scalar.activation(rstdrow[:], lvrow[:], AF.Exp, scale=-0.5)
                mrp = pB.tile([128, 2, N2], f32, name="mrp", tag="mrp")
                nc.tensor.matmul(mrp[:, 0, :], lhsT=ones_row[:], rhs=murow[:, 0, :],
                                 start=True, stop=True)
                nc.tensor.matmul(mrp[:, 1, :], lhsT=ones_row[:], rhs=rstdrow[:],
                                 start=True, stop=True)
                nc.vector.tensor_copy(murep[:], mrp[:, 0, :])
                nc.vector.tensor_copy(rsrep[:], mrp[:, 1, :])
                for mo in range(2):
                    nc.vector.tensor_tensor(xm[:, mo, :], cvo[:, mo, :], murep[:],
                                            ALU.subtract)
                    nc.vector.tensor_tensor(xn[:, mo, :], xm[:, mo, :], rsrep[:],
                                            ALU.mult)

            # ----- middle: table + d/relu/gather interleaved + y -----
            with (
                tc.tile_pool(name="mid", bufs=1) as mp,
                tc.tile_pool(name="pD", bufs=2, space="PSUM") as pD,
                tc.tile_pool(name="pW", bufs=1, space="PSUM") as pW,
                tc.tile_pool(name="pC", bufs=2, space="PSUM") as pC,
            ):
                bas = mp.tile([128, NCH, 1024], f16)

                # table build
                kps_k = pC.tile([1, N2], f32, name="kps_k", tag="tb")
                kps_v = pC.tile([1, N2], f32, name="kps_v", tag="tb")
                for mo in range(2):
                    nc.tensor.matmul(kps_k[:], lhsT=wkvsb[:, mo, 0:1],
                                     rhs=xn[:, mo, :], start=(mo == 0), stop=(mo == 1))
                    nc.tensor.matmul(kps_v[:], lhsT=wkvsb[:, mo, 1:2],
                                     rhs=xn[:, mo, :], start=(mo == 0), stop=(mo == 1))
                nc.vector.tensor_scalar(ks_r[:], kps_k[:], cb_r[0:1, 0:1], None,
                                        ALU.add)
                nc.vector.tensor_scalar(vs_r[:], kps_v[:], cb_r[0:1, 1:2], None,
                                        ALU.add)
                nc.vector.reduce_max(kmx[:], ks_r[:], axis=AX.X)
                nc.vector.tensor_reduce(kmn[:], ks_r[:], axis=AX.X, op=ALU.min)
                kmp = pC.tile([1, 2, N2], f32, name="kmp", tag="tb")
                nc.tensor.matmul(kmp[:, 0, :], lhsT=kmx[:], rhs=mones256[:],
                                 start=True, stop=True)
                nc.tensor.matmul(kmp[:, 1, :], lhsT=kmn[:], rhs=mones256[:],
                                 start=True, stop=True)
                nc.vector.tensor_scalar(kmsb[:], kmp[:], -1.0, None, ALU.mult)
                nc.vector.tensor_scalar(kmsb[:, 0, :], kmsb[:, 0, :], -1.0, None,
                                        ALU.mult)
                nc.vector.tensor_copy(krows[0:1, :], ks_r[:])
                nc.sync.dma_start(krows[1:3, :], kmsb[0:1, :, :])
                Xp = pC.tile([64, N2], f32, name="Xp", tag="tb")
                nc.tensor.matmul(Xp[:], lhsT=grid3, rhs=krows[:], start=True,
                                 stop=True)
                nc.scalar.activation(Esb[:], Xp[:], AF.Exp)
                vrp = pC.tile([64, N2], f32, name="vrp", tag="tb")
                nc.tensor.matmul(vrp[:], lhsT=ones_row[0:1, 0:64], rhs=vs_r[:],
                                 start=True, stop=True)
                nc.vector.tensor_tensor(Evm[:], Esb[:], vrp[:], ALU.mult)
                nc.vector.reduce_sum(numv[:], Evm[:], axis=AX.X)
                nc.vector.reduce_sum(denv[:], Esb[:], axis=AX.X)
                nc.vector.reciprocal_approx_fast(dinv[:], denv[:])
                nc.vector.tensor_tensor(Tcol[:], numv[:], dinv[:], ALU.mult)
                nc.vector.memset(Ts1[:], 0.0)
                nc.vector.memset(mcol[:], 0.0)
                nc.vector.memset(msh[:], 0.0)
                nc.scalar.dma_start(Ts1[0:62, :], Tcol[1:63, :])
                nc.vector.tensor_tensor(mcol[0:62, :], Ts1[0:62, :],
                                        Tcol[0:62, :], ALU.subtract)
                nc.sync.dma_start(msh[1:63, :], mcol[0:62, :])
                nc.vector.tensor_tensor(ccol[:], mcol[:], msh[:], ALU.subtract)
                nc.vector.tensor_scalar(msrow[:], msum_r, Tcol[0:1, 0:1], None,
                                        ALU.mult)
                rp2 = pC.tile([128, C], f32, name="rp2", tag="tb")
                nc.tensor.matmul(rp2[:], lhsT=ones_row[:], rhs=msrow[:],
                                 start=True, stop=True)
                nc.vector.tensor_tensor(bprep2[:], rp2[:], bprep[:], ALU.add)
                nc.sync.dma_start(cTrow[0:1, :], ccol[:, 0:1])
                cvp = pC.tile([64, 64], f32, name="cvp", tag="tb")
                nc.tensor.matmul(cvp[:], lhsT=cTrow[:], rhs=E8row, start=True,
                                 stop=True)
                nc.vector.tensor_copy(crepv[0:64, :], cvp[:])
                nc.vector.tensor_copy(crepv[64:128, :], cvp[:])

                # d + relu + gather interleaved; y per group at the end
                wp = pW.tile([128, 1024], f32)
                dp_q = {}

                def dmm(cc):
                    sp = 32 * (cc % 4)
                    dp = pD.tile([128, 1024], f32, name=f"dp{cc}", tag="dp")
                    for q in range(2):
                        for cq in range(2):
                            base = 2048 * cc + 1024 * q + 512 * cq
                            nc.tensor.matmul(
                                dp[64 * q:64 * q + 64, 512 * cq:512 * (cq + 1)],
                                lhsT=onesd[sp:sp + 1, 0:64],
                                rhs=trhs[sp:sp + 1, base:base + 512],
                                start=True, stop=True,
                                tile_position=(sp, 64 * q),
                            )
                    dp_q[cc] = dp

                def relu(cc):
                    dp = dp_q.pop(cc)
                    if cc % 2 == 0:
                        nc.scalar.activation(bas[:, cc, :], dp[:], AF.Relu,
                                             bias=miota)
                    else:
                        nc.vector.tensor_scalar(bas[:, cc, :], dp[:], miota, 0.0,
                                                ALU.add, ALU.max)

                def gmm(a, h):
                    nh, q = divmod(a, 2)
                    cc = 2 * h + nh
                    for cq in range(2):
                        nc.tensor.matmul(
                            wp[32 * a:32 * a + 8, 512 * cq:512 * (cq + 1)],
                            lhsT=crepv[64 * q:64 * q + 64, 8 * h:8 * h + 8],
                            rhs=bas[64 * q:64 * q + 64, cc,
                                    512 * cq:512 * (cq + 1)],
                            start=(h == 0), stop=(h == 7),
                            tile_position=(64 * q, 32 * a),
                            skip_group_check=True,
                        )

                for cc in range(4):
                    dmm(cc)
                    relu(cc)
                for h in range(8):
                    for cc in (2 * h + 4, 2 * h + 5):
                        if cc < NCH:
                            dmm(cc)
                            relu(cc)
                    for a in range(4):
                        gmm(a, h)

                with (
                    tc.tile_pool(name="ysq", bufs=4) as ysq,
                    tc.tile_pool(name="wgq", bufs=2) as wgq,
                ):
                    for a in range(4):
                        nh, q = divmod(a, 2)
                        wg = wgq.tile([8, 1024], f16, name=f"wg{a}", tag="wg")
                        nc.vector.tensor_copy(wg[:], wp[32 * a:32 * a + 8, :])
                        nbase = 2048 * nh + 1024 * q
                        eng = nc.sync if a % 2 == 0 else nc.scalar
                        eng.dma_start(wtsT[0:8, nbase:nbase + 1024], wg[:])
                        eng2 = nc.gpsimd if a % 2 == 0 else nc.sync
                        eng2.dma_start(wtsT[64:72, nbase:nbase + 1024], wg[:])
                        for j in range(8):
                            nb = (nh * 2 + q) * 8 + j
                            sp = 64 * (nb % 2)
                            yp = pC.tile([128, C], f32, name=f"yp{nb}", tag="tb")
                            nc.tensor.matmul(
                                yp[:], lhsT=wtsT[sp:sp + 8, 128 * nb:128 * (nb + 1)],
                                rhs=mm16[sp:sp + 8, :], start=True, stop=True,
                                tile_position=(sp, 0))
                            ysb = ysq.tile([128, C], f16, name=f"ysb{nb}",
                                           tag="ysb")
                            nc.vector.tensor_tensor(ysb[:], yp[:], bprep2[:],
                                                    ALU.add)
                            ydma = (nc.gpsimd.dma_start, nc.sync.dma_start,
                                    nc.scalar.dma_start)[nb % 3]
                            ydma(y_d[128 * nb:128 * (nb + 1), :], ysb[:])

                    if debug:
                        nc.sync.dma_start(
                            dbg_t[:],
                            trhs[0:1, :].rearrange("p (h n) -> p h n", h=8))
                        nc.sync.dma_start(dbg_T[:, 0:1], Tcol[:])
                        nc.sync.dma_start(dbg_T[:, 1:2], ccol[:])
                        nc.sync.dma_start(dbg_T[:, 2:3], mcol[:])
                        nc.sync.dma_start(dbg_T[:, 3:4], Ts1[:])
                        nc.sync.dma_start(dbg_kv[:, 0:N2], ks_r[:])
                        nc.sync.dma_start(dbg_kv[:, N2:2 * N2], vs_r[:])
                        nc.sync.dma_start(dbg_w[:], wtsT[0:8, :])
                        nc.sync.dma_start(dbg_xn[:, 0:N2], xn[:, 0, :])
                        nc.sync.dma_start(dbg_xn[:, N2:2 * N2], xn[:, 1, :])

    nc.compile()
    return nc


def _host_precompute(Wq, Wkv, Wsr, bsr, gamma, beta, Wproj, bproj, k_learn, v_learn):
    lksum = k_learn.reshape(HEADS, HC).sum(1)
    wqs = (Wq.reshape(C, HEADS, HC).sum(2) * (SCALE * lksum)[None, :]).astype(np.float32)
    wk = Wkv[:, 0::2].sum(1)
    wv = Wkv[:, 1::2].sum(1)
    wkv2 = np.stack([gamma * wk, gamma * wv], 1).astype(np.float32)
    lv = v_learn.reshape(HEADS, HC)
    Mmat = np.zeros((HEADS, C), np.float32)
    for h in range(HEADS):
        Mmat[h] = lv[h] @ Wproj[h::HEADS]
    Wsr_flat = np.ascontiguousarray(
        Wsr.transpose(2, 3, 1, 0).reshape(SR * SR * C, C)
    ).astype(np.float16)

    larr = np.arange(1, L + 1).astype(np.float64)
    larr[L - 1] = float(L - 1)
    g = BW * np.arctanh((larr - L / 2) / AMP)

    b32a = np.zeros((128, 8), np.float32)
    b32a[:, 0] = -((np.arange(128) % 64) + 1.0)
    b32a[:, 1:3] = bsr.reshape(2, 128).T
    b32a[:, 3:7] = wkv2.reshape(2, 128, 2).transpose(1, 0, 2).reshape(128, 4)

    b32b = np.zeros((4, 648), np.float32)
    b32b[0, 0:64] = g
    b32b[1, 0:64] = np.maximum(g, 0)
    b32b[2, 0:64] = np.maximum(-g, 0)
    e8 = np.zeros((8, 8), np.float32)
    np.fill_diagonal(e8, 1.0)
    b32b[0, 64:128] = e8.reshape(-1)
    b32b[0, 128:384] = Mmat.sum(0)
    b32b[0, 384:640] = bproj
    b32b[0, 640] = float((beta * wk).sum())
    b32b[0, 641] = float((beta * wv).sum())

    wq16 = wqs.reshape(2, 128, HEADS).transpose(1, 0, 2).reshape(128, 16).astype(np.float16)
    mm8 = Mmat.astype(np.float16)
    return dict(wsr=Wsr_flat, b32a=b32a, b32b=b32b, wq16=wq16, mm8=mm8)


def kernel(**inputs):
    x = np.asarray(inputs["x"], np.float32)
    weights = _host_precompute(
        *[np.asarray(inputs[k], np.float32) for k in
          ("Wq", "Wkv", "Wsr", "bsr", "gamma", "beta", "Wproj", "bproj",
           "k_learn", "v_learn")]
    )
    if "nc" not in _NC_CACHE:
        _NC_CACHE["nc"] = _build_nc()
    nc = _NC_CACHE["nc"]
    in_maps = [
        {"xt": np.ascontiguousarray(x[i].T).astype(np.float16), **weights}
        for i in range(B)
    ]
    from concourse.bass_utils import run_bass_kernel_spmd

    res = run_bass_kernel_spmd(nc, in_maps, core_ids=list(range(B)))
    y = np.stack([res.results[i]["y"].astype(np.float32) for i in range(B)], 0)
    return y


# revision 4
# speedup vs baseline: 1.0960x; 1.0913x over previous
"""Trainium2 Bass kernel for nn_Attention_17489106830121 (v3: L=64 stacked).

Same math as v2 (rank-1 logits -> per-core scalar function f(s) evaluated
by exact piecewise-linear interpolation on a warped grid), with:
  - L=64 knots (sim rel err 5.5e-3 vs 2e-2 tolerance), t = 32+31.3*tanh(s/3)
  - stacked layout: each [128, 1024] PSUM chunk holds TWO 64-row l-halves
    covering 2048 flat (h,n) columns -> relu pass halves (per-column cost)
  - 16 chunks; d = rank-1 broadcast matmuls (2 per chunk, col positions
    0/64, row strips 0/32/64/96); relu folds the knot offset via per-
    partition bias (ScalarE) / fused add,max (DVE), alternating engines
  - gather: 4 groups = (n-half nh, l-half q) at tile_position (64q, 32a),
    8 accumulating matmuls each (variant-8 lhsT, c placed at col h),
    interleaved with the d stream in the PE queue
  - blobs shrunk to ~22KB (b32a [128,8], b32b [4,640], wqs, mm8)
  - y per group right after its evac DMA
"""

import numpy as np

B, N, C, HEADS, SR = 8, 4096, 256, 8, 4
HC = C // HEADS
SCALE = HC ** -0.5
EPS = 1e-5
HS = 64 // SR
N2 = HS * HS             # 256
L = 64
BW = 3.0
AMP = 31.25              # exact in fp16; t = 32 + 31.25*tanh(s/3)
NCH = 16                 # [128,1024] stacked chunks; 2048 flat cols each

_NC_CACHE = {}


def _build_nc(debug=False):
    import concourse.bass as bass
    import concourse.bacc as bacc
    import concourse.mybir as mybir
    from concourse import tile

    dt = mybir.dt
    f32, f16 = dt.float32, dt.float16
    AF = mybir.ActivationFunctionType
    ALU = mybir.AluOpType
    AX = mybir.AxisListType

    nc = bacc.Bacc(None, target_bir_lowering=False)

    xt_d = nc.dram_tensor("xt", [2 * 128, N], f16, kind="ExternalInput")
    ws_d = nc.dram_tensor("wsr", [SR * SR * C, C], f16, kind="ExternalInput")
    b32a_d = nc.dram_tensor("b32a", [128, 8], f32, kind="ExternalInput")
    b32b_d = nc.dram_tensor("b32b", [4, 648], f32, kind="ExternalInput")
    wq_d = nc.dram_tensor("wq16", [128, 16], f16, kind="ExternalInput")
    mm_d = nc.dram_tensor("mm8", [8, C], f16, kind="ExternalInput")
    y_d = nc.dram_tensor("y", [N, C], f16, kind="ExternalOutput")
    if debug:
        dbg_t = nc.dram_tensor("dbg_t", [8, N], f16, kind="ExternalOutput")
        dbg_T = nc.dram_tensor("dbg_T", [64, 4], f32, kind="ExternalOutput")
        dbg_kv = nc.dram_tensor("dbg_kv", [1, 2 * N2], f32, kind="ExternalOutput")
        dbg_xn = nc.dram_tensor("dbg_xn", [128, 2 * N2], f32, kind="ExternalOutput")

    with tile.TileContext(nc) as tc:
        with tc.tile_pool(name="const", bufs=1) as cp:
            b32a = cp.tile([128, 8], f32)
            b32b = cp.tile([4, 648], f32)
            xt = cp.tile([128, 2, N], f16)
            wssb = cp.tile([128, 32, C], f16)
            trhs = cp.tile([97, 8 * N], f16)
            onesd = cp.tile([97, 128], f16)
            crepv = cp.tile([128, 64], f16)
            mm16 = cp.tile([73, C], f16)
            ones1k = cp.tile([1, 1024], f16)
            browf = cp.tile([1, C], f32)
            brow16 = cp.tile([1, C], f16)
            wqssb = cp.tile([128, 2, HEADS], f16)
            murep = cp.tile([128, N2], f32)
            rsrep = cp.tile([128, N2], f32)
            ones_row = cp.tile([1, 128], f32)
            ones_col = cp.tile([128, 1], f32)
            mones256 = cp.tile([1, N2], f32)
            eps_sb = cp.tile([128, 1], f32)
            cvo = cp.tile([128, 2, N2], f32)
            xm = cp.tile([128, 2, N2], f32)
            xn = cp.tile([128, 2, N2], f32)
            sq2 = cp.tile([128, 2, N2], f32)
            murow = cp.tile([1, 2, N2], f32)
            varrow = cp.tile([1, N2], f32)
            lvrow = cp.tile([1, N2], f32)
            rstdrow = cp.tile([1, N2], f32)
            ks_r = cp.tile([1, N2], f32)
            vs_r = cp.tile([1, N2], f32)
            kmx = cp.tile([1, 1], f32)
            kmn = cp.tile([1, 1], f32)
            kmsb = cp.tile([1, 2, N2], f32)
            krows = cp.tile([3, N2], f32)
            Esb = cp.tile([64, N2], f32)
            Evm = cp.tile([64, N2], f32)
            numv = cp.tile([64, 1], f32)
            denv = cp.tile([64, 1], f32)
            dinv = cp.tile([64, 1], f32)
            Tcol = cp.tile([64, 1], f32)
            Ts1 = cp.tile([64, 1], f32)
            mcol = cp.tile([64, 1], f32)
            msh = cp.tile([64, 1], f32)
            ccol = cp.tile([64, 1], f32)
            cTrow = cp.tile([1, 64], f32)
            msrow = cp.tile([1, C], f32)

            miota = b32a[:, 0:1]
            bsrcol = b32a[:, 1:3]
            wkvsb = b32a[:, 3:7].rearrange("p (t h) -> p t h", t=2)
            grid3 = b32b[0:3, 0:64]
            E8row = b32b[0:1, 64:128]
            msum_r = b32b[0:1, 128:384]
            bpr_r = b32b[0:1, 384:640]
            cb_r = b32b[0:1, 640:642]

            # ----- input DMAs: xt 6 pieces across all 3 queues, then wssb
            # staged in conv consumption order -----
            xd = xt_d[:].rearrange("(ct p) n -> p ct n", p=128)
            wsv = ws_d[:].rearrange("(t p) c -> p t c", p=128)
            nc.sync.dma_start(xt[:, 0, 0:2731], xd[:, 0, 0:2731])
            nc.scalar.dma_start(xt[:, 1, 0:2731], xd[:, 1, 0:2731])
            nc.gpsimd.dma_start(xt[:, 0, 2731:4096], xd[:, 0, 2731:4096])
            nc.gpsimd.dma_start(xt[:, 1, 2731:4096], xd[:, 1, 2731:4096])
            nc.gpsimd.dma_start(wqssb[:], wq_d[:].rearrange("p (t h) -> p t h", t=2))
            nc.gpsimd.dma_start(b32a[:], b32a_d[:])
            nc.gpsimd.dma_start(wssb[:, 0:8, :], wsv[:, 0:8, :])
            nc.sync.dma_start(wssb[:, 8:16, :], wsv[:, 8:16, :])
            nc.scalar.dma_start(wssb[:, 16:24, :], wsv[:, 16:24, :])
            nc.sync.dma_start(wssb[:, 24:28, :], wsv[:, 24:28, :])
            nc.scalar.dma_start(wssb[:, 28:32, :], wsv[:, 28:32, :])
            nc.gpsimd.dma_start(b32b[:], b32b_d[:])
            nc.gpsimd.dma_start(mm16[0:8, :], mm_d[:])
            nc.gpsimd.dma_start(mm16[64:72, :], mm_d[:])
            nc.vector.memset(ones_row[:], 1.0)
            nc.vector.memset(mones256[:], -1.0)
            nc.vector.memset(eps_sb[:], EPS)
            nc.vector.memset(ones_col[:], 1.0)
            nc.vector.memset(ones1k[:], 1.0)
            for sp in (0, 32, 64, 96):
                nc.vector.memset(onesd[sp:sp + 1, :], AMP)

            # ----- s^T, warp to t, flatten; conv/LN shares the PSUM scope --
            with (
                tc.tile_pool(name="ssp", bufs=1) as ssp,
                tc.tile_pool(name="psSB", bufs=2, space="PSUM") as pB,
            ):
                t16 = ssp.tile([8, N], f16)
                if True:
                    pS = pB
                    for k4 in range(2):
                        sps = pS.tile([128, 512], f32, name="sps", tag="sps")
                        for q in range(4):
                            k = 4 * k4 + q
                            cb = 32 * q
                            for ct in range(2):
                                nc.tensor.matmul(
                                    sps[cb:cb + 8, :],
                                    lhsT=wqssb[:, ct, :],
                                    rhs=xt[:, ct, 512 * k:512 * (k + 1)],
                                    start=(ct == 0), stop=(ct == 1),
                                    tile_position=(0, cb),
                                )
                        for q in range(4):
                            k = 4 * k4 + q
                            sl = slice(512 * k, 512 * (k + 1))
                            nc.scalar.activation(t16[:, sl],
                                                 sps[32 * q:32 * q + 8, :],
                                                 AF.Tanh, scale=1.0 / BW)
                    for k in range(8):
                        sl = slice(512 * k, 512 * (k + 1))
                        for i, sp in enumerate((0, 32, 64, 96)):
                            eng2 = (nc.sync, nc.scalar, nc.gpsimd,
                                    nc.sync)[(4 * k + i) % 4]
                            eng2.dma_start(
                                trhs[sp:sp + 1, :].rearrange(
                                    "p (h n) -> p h n", h=8)[:, :, sl],
                                t16[:, sl],
                            )
                # ----- conv (out [c_out, spatial]) + LN stats -----
                xtr = xt[:].rearrange("p ct (ph kh pw kw) -> p ct kh kw ph pw",
                                      ph=16, kh=4, pw=16, kw=4)
                for mo in range(2):
                    cpsA = pB.tile([128, N2], f32, name=f"cpsA{mo}", tag="cpsA",
                                   bufs=1)
                    cpsB = pB.tile([128, N2], f32, name=f"cpsB{mo}", tag="cpsB",
                                   bufs=1)
                    for kh in range(4):
                        for kw in range(4):
                            for ct in range(2):
                                kidx = kh * 8 + kw * 2 + ct
                                cnt = kh * 8 + kw * 2 + ct
                                nc.tensor.matmul(
                                    cpsA[:],
                                    lhsT=wssb[0:64, kidx, 128 * mo:128 * (mo + 1)],
                                    rhs=xtr[0:64, ct, kh, kw],
                                    start=(cnt == 0), stop=(cnt == 31),
                                    tile_position=(0, 0),
                                )
                                nc.tensor.matmul(
                                    cpsB[:],
                                    lhsT=wssb[64:128, kidx, 128 * mo:128 * (mo + 1)],
                                    rhs=xtr[64:128, ct, kh, kw],
                                    start=(cnt == 0), stop=(cnt == 31),
                                    tile_position=(64, 0),
                                )
                    nc.vector.tensor_scalar(cvo[:, mo, :], cpsA[:],
                                            bsrcol[:, mo:mo + 1], None, ALU.add)
                    nc.vector.tensor_tensor(cvo[:, mo, :], cvo[:, mo, :],
                                            cpsB[:], ALU.add)
                nc.vector.tensor_tensor(sq2[:], cvo[:], cvo[:], ALU.mult)
                muA = pB.tile([1, N2], f32, name="muA", tag="muA", bufs=1)
                muB = pB.tile([1, N2], f32, name="muB", tag="muB", bufs=1)
                for mo in range(2):
                    nc.tensor.matmul(muA[:], lhsT=ones_col[:],
                                     rhs=cvo[:, mo, :], start=(mo == 0),
                                     stop=(mo == 1))
                    nc.tensor.matmul(muB[:], lhsT=ones_col[:],
                                     rhs=sq2[:, mo, :], start=(mo == 0),
                                     stop=(mo == 1))
                nc.vector.tensor_scalar(murow[:, 0, :], muA[:], 1.0 / N2, None,
                                        ALU.mult)
                nc.vector.tensor_scalar(murow[:, 1, :], muB[:], 1.0 / N2, None,
                                        ALU.mult)
                nc.vector.tensor_tensor(varrow[:], murow[:, 0, :], murow[:, 0, :],
                                        ALU.mult)
                nc.vector.tensor_tensor(varrow[:], murow[:, 1, :], varrow[:],
                                        ALU.subtract)
                nc.scalar.activation(lvrow[:], varrow[:], AF.Ln,
                                     bias=eps_sb[0:1, :])
                nc.scalar.activation(rstdrow[:], lvrow[:], AF.Exp, scale=-0.5)
                mrp = pB.tile([128, 2, N2], f32, name="mrp", tag="mrp", bufs=1)
                nc.tensor.matmul(mrp[:, 0, :], lhsT=ones_row[:], rhs=murow[:, 0, :],
                                 start=True, stop=True)
                nc.tensor.matmul(mrp[:, 1, :], lhsT=ones_row[:], rhs=rstdrow[:],
                                 start=True, stop=True)
                nc.vector.tensor_copy(murep[:], mrp[:, 0, :])
                nc.vector.tensor_copy(rsrep[:], mrp[:, 1, :])
                for mo in range(2):
                    nc.vector.tensor_tensor(xm[:, mo, :], cvo[:, mo, :], murep[:],
                                            ALU.subtract)
                    nc.vector.tensor_tensor(xn[:, mo, :], xm[:, mo, :], rsrep[:],
                                            ALU.mult)

            # ----- middle: table + d/relu/gather interleaved + y -----
            with (
                tc.tile_pool(name="mid", bufs=1) as mp,
                tc.tile_pool(name="pD", bufs=2, space="PSUM") as pD,
                tc.tile_pool(name="pW", bufs=1, space="PSUM") as pW,
                tc.tile_pool(name="pC", bufs=2, space="PSUM") as pC,
            ):
                bas = mp.tile([128, NCH, 1024], f16)

                # table build
                kps_k = pC.tile([1, N2], f32, name="kps_k", tag="tb")
                kps_v = pC.tile([1, N2], f32, name="kps_v", tag="tb")
                for mo in range(2):
                    nc.tensor.matmul(kps_k[:], lhsT=wkvsb[:, mo, 0:1],
                                     rhs=xn[:, mo, :], start=(mo == 0), stop=(mo == 1))
                    nc.tensor.matmul(kps_v[:], lhsT=wkvsb[:, mo, 1:2],
                                     rhs=xn[:, mo, :], start=(mo == 0), stop=(mo == 1))
                nc.vector.tensor_scalar(ks_r[:], kps_k[:], cb_r[0:1, 0:1], None,
                                        ALU.add)
                nc.vector.tensor_scalar(vs_r[:], kps_v[:], cb_r[0:1, 1:2], None,
                                        ALU.add)
                nc.vector.reduce_max(kmx[:], ks_r[:], axis=AX.X)
                nc.vector.tensor_reduce(kmn[:], ks_r[:], axis=AX.X, op=ALU.min)
                kmp = pC.tile([1, 2, N2], f32, name="kmp", tag="tb")
                nc.tensor.matmul(kmp[:, 0, :], lhsT=kmx[:], rhs=mones256[:],
                                 start=True, stop=True)
                nc.tensor.matmul(kmp[:, 1, :], lhsT=kmn[:], rhs=mones256[:],
                                 start=True, stop=True)
                nc.vector.tensor_scalar(kmsb[:], kmp[:], -1.0, None, ALU.mult)
                nc.vector.tensor_scalar(kmsb[:, 0, :], kmsb[:, 0, :], -1.0, None,
                                        ALU.mult)
                nc.vector.tensor_copy(krows[0:1, :], ks_r[:])
                nc.sync.dma_start(krows[1:3, :], kmsb[0:1, :, :])
                Xp = pC.tile([64, N2], f32, name="Xp", tag="tb")
                nc.tensor.matmul(Xp[:], lhsT=grid3, rhs=krows[:], start=True,
                                 stop=True)
                nc.scalar.activation(Esb[:], Xp[:], AF.Exp)
                vrp = pC.tile([64, N2], f32, name="vrp", tag="tb")
                nc.tensor.matmul(vrp[:], lhsT=ones_row[0:1, 0:64], rhs=vs_r[:],
                                 start=True, stop=True)
                nc.vector.tensor_tensor(Evm[:], Esb[:], vrp[:], ALU.mult)
                nc.vector.reduce_sum(numv[:], Evm[:], axis=AX.X)
                nc.vector.reduce_sum(denv[:], Esb[:], axis=AX.X)
                nc.vector.reciprocal_approx_fast(dinv[:], denv[:])
                nc.vector.tensor_tensor(Tcol[:], numv[:], dinv[:], ALU.mult)
                nc.vector.memset(Ts1[:], 0.0)
                nc.vector.memset(mcol[:], 0.0)
                nc.vector.memset(msh[:], 0.0)
                nc.scalar.dma_start(Ts1[0:62, :], Tcol[1:63, :])
                nc.vector.tensor_tensor(mcol[0:62, :], Ts1[0:62, :],
                                        Tcol[0:62, :], ALU.subtract)
                nc.sync.dma_start(msh[1:63, :], mcol[0:62, :])
                nc.vector.tensor_tensor(ccol[:], mcol[:], msh[:], ALU.subtract)
                nc.vector.tensor_scalar(msrow[:], msum_r, Tcol[0:1, 0:1], None,
                                        ALU.mult)
                nc.vector.tensor_tensor(browf[:], msrow[:], bpr_r, ALU.add)
                nc.vector.tensor_copy(brow16[:], browf[:])
                nc.sync.dma_start(mm16[8:9, :], brow16[:])
                nc.scalar.dma_start(mm16[72:73, :], brow16[:])
                nc.sync.dma_start(cTrow[0:1, :], ccol[:, 0:1])
                cvp = pC.tile([64, 64], f32, name="cvp", tag="tb")
                nc.tensor.matmul(cvp[:], lhsT=cTrow[:], rhs=E8row, start=True,
                                 stop=True)
                nc.vector.tensor_copy(crepv[0:64, :], cvp[:])
                nc.vector.tensor_copy(crepv[64:128, :], cvp[:])

                # d + relu + gather; evens feed groups 0/1, odds 2/3;
                # y for groups 0/1 overlaps the odd half
                wp = pW.tile([128, 1024], f32)
                dp_q = {}

                def dmm(cc):
                    sp = 32 * (cc % 4)
                    dp = pD.tile([128, 1024], f32, name=f"dp{cc}", tag="dp")
                    for q in range(2):
                        for cq in range(2):
                            base = 2048 * cc + 1024 * q + 512 * cq
                            nc.tensor.matmul(
                                dp[64 * q:64 * q + 64, 512 * cq:512 * (cq + 1)],
                                lhsT=onesd[sp:sp + 1, 0:64],
                                rhs=trhs[sp:sp + 1, base:base + 512],
                                start=True, stop=True,
                                tile_position=(sp, 64 * q),
                            )
                    dp_q[cc] = dp

                def relu(cc):
                    dp = dp_q.pop(cc)
                    if cc % 2 == 0:
                        nc.scalar.activation(bas[:, cc, :], dp[:], AF.Relu,
                                             bias=miota)
                    else:
                        nc.vector.tensor_scalar(bas[:, cc, :], dp[:], miota, 0.0,
                                                ALU.add, ALU.max)

                def gmm(a, h):
                    nh, q = divmod(a, 2)
                    cc = 2 * h + nh
                    for cq in range(2):
                        nc.tensor.matmul(
                            wp[32 * a:32 * a + 8, 512 * cq:512 * (cq + 1)],
                            lhsT=crepv[64 * q:64 * q + 64, 8 * h:8 * h + 8],
                            rhs=bas[64 * q:64 * q + 64, cc,
                                    512 * cq:512 * (cq + 1)],
                            start=(h == 0), stop=(h == 7),
                            tile_position=(64 * q, 32 * a),
                            skip_group_check=True,
                        )

                with (
                    tc.tile_pool(name="ysq", bufs=4) as ysq,
                    tc.tile_pool(name="wgq", bufs=2) as wgq,
                ):
                    wgs = {}

                    def evac(a):
                        wg = wgq.tile([73, 1024], f16, name=f"wg{a}", tag="wg")
                        nc.vector.tensor_copy(wg[0:8, :], wp[32 * a:32 * a + 8, :])
                        nc.scalar.copy(wg[64:72, :], wp[32 * a:32 * a + 8, :])
                        nc.gpsimd.dma_start(wg[8:9, :], ones1k[:])
                        nc.sync.dma_start(wg[72:73, :], ones1k[:])
                        wgs[a] = wg

                    def ymm(a, j):
                        nb = 8 * a + j
                        sp = 64 * ((a + j) % 2)
                        yp = pC.tile([128, C], f32, name=f"yp{nb}", tag="tb")
                        nc.tensor.matmul(
                            yp[:], lhsT=wgs[a][sp:sp + 9, 128 * j:128 * (j + 1)],
                            rhs=mm16[sp:sp + 9, :], start=True, stop=True,
                            tile_position=(sp, 0))
                        ysb = ysq.tile([128, C], f16, name=f"ysb{nb}", tag="ysb")
                        if nb % 2 == 0:
                            nc.vector.tensor_copy(ysb[:], yp[:])
                        else:
                            nc.scalar.copy(ysb[:], yp[:])
                        ydma = (nc.gpsimd.dma_start, nc.sync.dma_start,
                                nc.scalar.dma_start)[nb % 3]
                        ydma(y_d[128 * nb:128 * (nb + 1), :], ysb[:])

                    for cc in range(NCH):
                        dmm(cc)
                        relu(cc)
                    for h in range(8):
                        gmm(0, h)
                        gmm(1, h)
                    evac(0)
                    evac(1)
                    for h in range(8):
                        gmm(2, h)
                        gmm(3, h)
                        ymm(0, h)
                        ymm(1, h)
                    evac(2)
                    evac(3)
                    for h in range(8):
                        ymm(2, h)
                        ymm(3, h)

                    if debug:
                        nc.sync.dma_start(
                            dbg_t[:],
                            trhs[0:1, :].rearrange("p (h n) -> p h n", h=8))
                        nc.sync.dma_start(dbg_T[:, 0:1], Tcol[:])
                        nc.sync.dma_start(dbg_T[:, 1:2], ccol[:])
                        nc.sync.dma_start(dbg_T[:, 2:3], mcol[:])
                        nc.sync.dma_start(dbg_T[:, 3:4], Ts1[:])
                        nc.sync.dma_start(dbg_kv[:, 0:N2], ks_r[:])
                        nc.sync.dma_start(dbg_kv[:, N2:2 * N2], vs_r[:])
                        nc.sync.dma_start(dbg_xn[:, 0:N2], xn[:, 0, :])
                        nc.sync.dma_start(dbg_xn[:, N2:2 * N2], xn[:, 1, :])

    nc.compile()
    return nc


def _host_precompute(Wq, Wkv, Wsr, bsr, gamma, beta, Wproj, bproj, k_learn, v_learn):
    lksum = k_learn.reshape(HEADS, HC).sum(1)
    wqs = (Wq.reshape(C, HEADS, HC).sum(2) * (SCALE * lksum)[None, :]).astype(np.float32)
    wk = Wkv[:, 0::2].sum(1)
    wv = Wkv[:, 1::2].sum(1)
    wkv2 = np.stack([gamma * wk, gamma * wv], 1).astype(np.float32)
    lv = v_learn.reshape(HEADS, HC)
    Mmat = np.zeros((HEADS, C), np.float32)
    for h in range(HEADS):
        Mmat[h] = lv[h] @ Wproj[h::HEADS]
    Wsr_flat = np.ascontiguousarray(
        Wsr.transpose(2, 3, 1, 0).reshape(SR * SR * C, C)
    ).astype(np.float16)

    larr = np.arange(1, L + 1).astype(np.float64)
    larr[L - 1] = float(L - 1)
    g = BW * np.arctanh((larr - L / 2) / AMP)

    b32a = np.zeros((128, 8), np.float32)
    b32a[:, 0] = float(L // 2) - ((np.arange(128) % 64) + 1.0)
    b32a[:, 1:3] = bsr.reshape(2, 128).T
    b32a[:, 3:7] = wkv2.reshape(2, 128, 2).transpose(1, 0, 2).reshape(128, 4)

    b32b = np.zeros((4, 648), np.float32)
    b32b[0, 0:64] = g
    b32b[1, 0:64] = np.maximum(g, 0)
    b32b[2, 0:64] = np.maximum(-g, 0)
    e8 = np.zeros((8, 8), np.float32)
    np.fill_diagonal(e8, 1.0)
    b32b[0, 64:128] = e8.reshape(-1)
    b32b[0, 128:384] = Mmat.sum(0)
    b32b[0, 384:640] = bproj
    b32b[0, 640] = float((beta * wk).sum())
    b32b[0, 641] = float((beta * wv).sum())

    wq16 = wqs.reshape(2, 128, HEADS).transpose(1, 0, 2).reshape(128, 16).astype(np.float16)
    mm8 = Mmat.astype(np.float16)
    return dict(wsr=Wsr_flat, b32a=b32a, b32b=b32b, wq16=wq16, mm8=mm8)


def kernel(**inputs):
    x = np.asarray(inputs["x"], np.float32)
    weights = _host_precompute(
        *[np.asarray(inputs[k], np.float32) for k in
          ("Wq", "Wkv", "Wsr", "bsr", "gamma", "beta", "Wproj", "bproj",
           "k_learn", "v_learn")]
    )
    if "nc" not in _NC_CACHE:
        _NC_CACHE["nc"] = _build_nc()
    nc = _NC_CACHE["nc"]
    in_maps = [
        {"xt": np.ascontiguousarray(x[i].T).astype(np.float16), **weights}
        for i in range(B)
    ]
    from concourse.bass_utils import run_bass_kernel_spmd

    res = run_bass_kernel_spmd(nc, in_maps, core_ids=list(range(B)))
    y = np.stack([res.results[i]["y"].astype(np.float32) for i in range(B)], 0)
    return y


# revision 5
# speedup vs baseline: 1.1148x; 1.0171x over previous
"""Trainium2 Bass kernel for nn_Attention_17489106830121 (v3: L=64 stacked).

Same math as v2 (rank-1 logits -> per-core scalar function f(s) evaluated
by exact piecewise-linear interpolation on a warped grid), with:
  - L=64 knots (sim rel err 5.5e-3 vs 2e-2 tolerance), t = 32+31.3*tanh(s/3)
  - stacked layout: each [128, 1024] PSUM chunk holds TWO 64-row l-halves
    covering 2048 flat (h,n) columns -> relu pass halves (per-column cost)
  - 16 chunks; d = rank-1 broadcast matmuls (2 per chunk, col positions
    0/64, row strips 0/32/64/96); relu folds the knot offset via per-
    partition bias (ScalarE) / fused add,max (DVE), alternating engines
  - gather: 4 groups = (n-half nh, l-half q) at tile_position (64q, 32a),
    8 accumulating matmuls each (variant-8 lhsT, c placed at col h),
    interleaved with the d stream in the PE queue
  - blobs shrunk to ~22KB (b32a [128,8], b32b [4,640], wqs, mm8)
  - y per group right after its evac DMA
"""

import numpy as np

B, N, C, HEADS, SR = 8, 4096, 256, 8, 4
HC = C // HEADS
SCALE = HC ** -0.5
EPS = 1e-5
HS = 64 // SR
N2 = HS * HS             # 256
L = 64
BW = 3.0
AMP = 31.25              # exact in fp16; t = 32 + 31.25*tanh(s/3)
NCH = 16                 # [128,1024] stacked chunks; 2048 flat cols each

_NC_CACHE = {}


def _build_nc(debug=False):
    import concourse.bass as bass
    import concourse.bacc as bacc
    import concourse.mybir as mybir
    from concourse import tile

    dt = mybir.dt
    f32, f16 = dt.float32, dt.float16
    AF = mybir.ActivationFunctionType
    ALU = mybir.AluOpType
    AX = mybir.AxisListType

    nc = bacc.Bacc(None, target_bir_lowering=False)

    xt_d = nc.dram_tensor("xt", [2 * 128, N], f16, kind="ExternalInput")
    ws_d = nc.dram_tensor("wsr", [SR * SR * C, C], f16, kind="ExternalInput")
    b32a_d = nc.dram_tensor("b32a", [128, 8], f32, kind="ExternalInput")
    b32b_d = nc.dram_tensor("b32b", [4, 648], f32, kind="ExternalInput")
    wq_d = nc.dram_tensor("wq16", [128, 16], f16, kind="ExternalInput")
    mm_d = nc.dram_tensor("mm8", [8, C], f16, kind="ExternalInput")
    y_d = nc.dram_tensor("y", [N, C], f16, kind="ExternalOutput")
    if debug:
        dbg_t = nc.dram_tensor("dbg_t", [8, N], f16, kind="ExternalOutput")
        dbg_T = nc.dram_tensor("dbg_T", [64, 4], f32, kind="ExternalOutput")
        dbg_kv = nc.dram_tensor("dbg_kv", [1, 2 * N2], f32, kind="ExternalOutput")
        dbg_xn = nc.dram_tensor("dbg_xn", [128, 2 * N2], f32, kind="ExternalOutput")

    with tile.TileContext(nc) as tc:
        with tc.tile_pool(name="const", bufs=1) as cp:
            b32a = cp.tile([128, 8], f32)
            b32b = cp.tile([4, 648], f32)
            xt = cp.tile([128, 2, N], f16)
            wssb = cp.tile([128, 32, C], f16)
            trhs = cp.tile([97, 8 * N], f16)
            onesd = cp.tile([97, 128], f16)
            crepv = cp.tile([128, 64], f16)
            mm16 = cp.tile([73, C], f16)
            ones1k = cp.tile([1, 1024], f16)
            browf = cp.tile([1, C], f32)
            brow16 = cp.tile([1, C], f16)
            wqssb = cp.tile([128, 2, HEADS], f16)
            murep = cp.tile([128, N2], f32)
            rsrep = cp.tile([128, N2], f32)
            ones_row = cp.tile([1, 128], f32)
            ones_col = cp.tile([128, 1], f32)
            mones256 = cp.tile([1, N2], f32)
            eps_sb = cp.tile([128, 1], f32)
            cvo = cp.tile([128, 2, N2], f32)
            xm = cp.tile([128, 2, N2], f32)
            xn = cp.tile([128, 2, N2], f32)
            sq2 = cp.tile([128, 2, N2], f32)
            murow = cp.tile([1, 2, N2], f32)
            varrow = cp.tile([1, N2], f32)
            lvrow = cp.tile([1, N2], f32)
            rstdrow = cp.tile([1, N2], f32)
            ks_r = cp.tile([1, N2], f32)
            vs_r = cp.tile([1, N2], f32)
            kmx = cp.tile([1, 1], f32)
            kmn = cp.tile([1, 1], f32)
            kmsb = cp.tile([1, 2, N2], f32)
            krows = cp.tile([3, N2], f32)
            Esb = cp.tile([64, N2], f32)
            Evm = cp.tile([64, N2], f32)
            numv = cp.tile([64, 1], f32)
            denv = cp.tile([64, 1], f32)
            dinv = cp.tile([64, 1], f32)
            Tcol = cp.tile([64, 1], f32)
            Ts1 = cp.tile([64, 1], f32)
            mcol = cp.tile([64, 1], f32)
            msh = cp.tile([64, 1], f32)
            ccol = cp.tile([64, 1], f32)
            cTrow = cp.tile([1, 64], f32)
            msrow = cp.tile([1, C], f32)

            miota = b32a[:, 0:1]
            bsrcol = b32a[:, 1:3]
            wkvsb = b32a[:, 3:7].rearrange("p (t h) -> p t h", t=2)
            grid3 = b32b[0:3, 0:64]
            E8row = b32b[0:1, 64:128]
            msum_r = b32b[0:1, 128:384]
            bpr_r = b32b[0:1, 384:640]
            cb_r = b32b[0:1, 640:642]

            # ----- input DMAs: xt 6 pieces across all 3 queues, then wssb
            # staged in conv consumption order -----
            xd = xt_d[:].rearrange("(ct p) n -> p ct n", p=128)
            wsv = ws_d[:].rearrange("(t p) c -> p t c", p=128)
            nc.sync.dma_start(xt[:, 0, 0:2731], xd[:, 0, 0:2731])
            nc.scalar.dma_start(xt[:, 1, 0:2731], xd[:, 1, 0:2731])
            nc.gpsimd.dma_start(xt[:, 0, 2731:4096], xd[:, 0, 2731:4096])
            nc.gpsimd.dma_start(xt[:, 1, 2731:4096], xd[:, 1, 2731:4096])
            nc.gpsimd.dma_start(wqssb[:], wq_d[:].rearrange("p (t h) -> p t h", t=2))
            nc.gpsimd.dma_start(b32a[:], b32a_d[:])
            nc.gpsimd.dma_start(wssb[:, 0:8, :], wsv[:, 0:8, :])
            nc.sync.dma_start(wssb[:, 8:16, :], wsv[:, 8:16, :])
            nc.scalar.dma_start(wssb[:, 16:24, :], wsv[:, 16:24, :])
            nc.sync.dma_start(wssb[:, 24:28, :], wsv[:, 24:28, :])
            nc.scalar.dma_start(wssb[:, 28:32, :], wsv[:, 28:32, :])
            nc.gpsimd.dma_start(b32b[:], b32b_d[:])
            nc.gpsimd.dma_start(mm16[0:8, :], mm_d[:])
            nc.gpsimd.dma_start(mm16[64:72, :], mm_d[:])
            nc.vector.memset(ones_row[:], 1.0)
            nc.vector.memset(mones256[:], -1.0)
            nc.vector.memset(eps_sb[:], EPS)
            nc.vector.memset(ones_col[:], 1.0)
            nc.vector.memset(ones1k[:], 1.0)
            for sp in (0, 32, 64, 96):
                nc.vector.memset(onesd[sp:sp + 1, :], AMP)

            # ----- s^T, warp to t, flatten; conv/LN shares the PSUM scope --
            with (
                tc.tile_pool(name="ssp", bufs=1) as ssp,
                tc.tile_pool(name="psSB", bufs=2, space="PSUM") as pB,
            ):
                t16 = ssp.tile([8, N], f16)
                if True:
                    pS = pB
                    for k4 in range(2):
                        sps = pS.tile([128, 512], f32, name="sps", tag="sps")
                        for q in range(4):
                            k = 4 * k4 + q
                            cb = 32 * q
                            for ct in range(2):
                                nc.tensor.matmul(
                                    sps[cb:cb + 8, :],
                                    lhsT=wqssb[:, ct, :],
                                    rhs=xt[:, ct, 512 * k:512 * (k + 1)],
                                    start=(ct == 0), stop=(ct == 1),
                                    tile_position=(0, cb),
                                )
                        for q in range(4):
                            k = 4 * k4 + q
                            sl = slice(512 * k, 512 * (k + 1))
                            nc.scalar.activation(t16[:, sl],
                                                 sps[32 * q:32 * q + 8, :],
                                                 AF.Tanh, scale=1.0 / BW)
                    for k in range(8):
                        sl = slice(512 * k, 512 * (k + 1))
                        for i, sp in enumerate((0, 32, 64, 96)):
                            eng2 = (nc.sync, nc.scalar, nc.gpsimd,
                                    nc.sync)[(4 * k + i) % 4]
                            eng2.dma_start(
                                trhs[sp:sp + 1, :].rearrange(
                                    "p (h n) -> p h n", h=8)[:, :, sl],
                                t16[:, sl],
                            )
                # ----- conv (out [c_out, spatial]) + LN stats -----
                xtr = xt[:].rearrange("p ct (ph kh pw kw) -> p ct kh kw ph pw",
                                      ph=16, kh=4, pw=16, kw=4)
                for mo in range(2):
                    cpsA = pB.tile([128, N2], f32, name=f"cpsA{mo}", tag="cpsA",
                                   bufs=1)
                    cpsB = pB.tile([128, N2], f32, name=f"cpsB{mo}", tag="cpsB",
                                   bufs=1)
                    for kh in range(4):
                        for kw in range(4):
                            for ct in range(2):
                                kidx = kh * 8 + kw * 2 + ct
                                cnt = kh * 8 + kw * 2 + ct
                                nc.tensor.matmul(
                                    cpsA[:],
                                    lhsT=wssb[0:64, kidx, 128 * mo:128 * (mo + 1)],
                                    rhs=xtr[0:64, ct, kh, kw],
                                    start=(cnt == 0), stop=(cnt == 31),
                                    tile_position=(0, 0),
                                )
                                nc.tensor.matmul(
                                    cpsB[:],
                                    lhsT=wssb[64:128, kidx, 128 * mo:128 * (mo + 1)],
                                    rhs=xtr[64:128, ct, kh, kw],
                                    start=(cnt == 0), stop=(cnt == 31),
                                    tile_position=(64, 0),
                                )
                    nc.vector.tensor_scalar(cvo[:, mo, :], cpsA[:],
                                            bsrcol[:, mo:mo + 1], None, ALU.add)
                    nc.vector.tensor_tensor(cvo[:, mo, :], cvo[:, mo, :],
                                            cpsB[:], ALU.add)
                nc.vector.tensor_tensor(sq2[:], cvo[:], cvo[:], ALU.mult)
                muA = pB.tile([1, N2], f32, name="muA", tag="muA", bufs=1)
                muB = pB.tile([1, N2], f32, name="muB", tag="muB", bufs=1)
                for mo in range(2):
                    nc.tensor.matmul(muA[:], lhsT=ones_col[:],
                                     rhs=cvo[:, mo, :], start=(mo == 0),
                                     stop=(mo == 1))
                    nc.tensor.matmul(muB[:], lhsT=ones_col[:],
                                     rhs=sq2[:, mo, :], start=(mo == 0),
                                     stop=(mo == 1))
                nc.vector.tensor_scalar(murow[:, 0, :], muA[:], 1.0 / N2, None,
                                        ALU.mult)
                nc.vector.tensor_scalar(murow[:, 1, :], muB[:], 1.0 / N2, None,
                                        ALU.mult)
                nc.vector.tensor_tensor(varrow[:], murow[:, 0, :], murow[:, 0, :],
                                        ALU.mult)
                nc.vector.tensor_tensor(varrow[:], murow[:, 1, :], varrow[:],
                                        ALU.subtract)
                nc.scalar.activation(lvrow[:], varrow[:], AF.Ln,
                                     bias=eps_sb[0:1, :])
                nc.scalar.activation(rstdrow[:], lvrow[:], AF.Exp, scale=-0.5)
                mrp = pB.tile([128, 2, N2], f32, name="mrp", tag="mrp", bufs=1)
                nc.tensor.matmul(mrp[:, 0, :], lhsT=ones_row[:], rhs=murow[:, 0, :],
                                 start=True, stop=True)
                nc.tensor.matmul(mrp[:, 1, :], lhsT=ones_row[:], rhs=rstdrow[:],
                                 start=True, stop=True)
                nc.vector.tensor_copy(murep[:], mrp[:, 0, :])
                nc.vector.tensor_copy(rsrep[:], mrp[:, 1, :])
                for mo in range(2):
                    nc.vector.tensor_tensor(xm[:, mo, :], cvo[:, mo, :], murep[:],
                                            ALU.subtract)
                    nc.vector.tensor_tensor(xn[:, mo, :], xm[:, mo, :], rsrep[:],
                                            ALU.mult)

            # ----- middle: table + d/relu/gather interleaved + y -----
            with (
                tc.tile_pool(name="mid", bufs=1) as mp,
                tc.tile_pool(name="pD", bufs=2, space="PSUM") as pD,
                tc.tile_pool(name="pW", bufs=1, space="PSUM") as pW,
                tc.tile_pool(name="pC", bufs=2, space="PSUM") as pC,
            ):
                bas = mp.tile([128, NCH, 1024], f16)

                # table build
                kps_k = pC.tile([1, N2], f32, name="kps_k", tag="tb")
                kps_v = pC.tile([1, N2], f32, name="kps_v", tag="tb")
                for mo in range(2):
                    nc.tensor.matmul(kps_k[:], lhsT=wkvsb[:, mo, 0:1],
                                     rhs=xn[:, mo, :], start=(mo == 0), stop=(mo == 1))
                    nc.tensor.matmul(kps_v[:], lhsT=wkvsb[:, mo, 1:2],
                                     rhs=xn[:, mo, :], start=(mo == 0), stop=(mo == 1))
                nc.vector.tensor_scalar(ks_r[:], kps_k[:], cb_r[0:1, 0:1], None,
                                        ALU.add)
                nc.vector.tensor_scalar(vs_r[:], kps_v[:], cb_r[0:1, 1:2], None,
                                        ALU.add)
                nc.vector.reduce_max(kmx[:], ks_r[:], axis=AX.X)
                nc.vector.tensor_reduce(kmn[:], ks_r[:], axis=AX.X, op=ALU.min)
                kmp = pC.tile([1, 2, N2], f32, name="kmp", tag="tb")
                nc.tensor.matmul(kmp[:, 0, :], lhsT=kmx[:], rhs=mones256[:],
                                 start=True, stop=True)
                nc.tensor.matmul(kmp[:, 1, :], lhsT=kmn[:], rhs=mones256[:],
                                 start=True, stop=True)
                nc.vector.tensor_scalar(kmsb[:], kmp[:], -1.0, None, ALU.mult)
                nc.vector.tensor_scalar(kmsb[:, 0, :], kmsb[:, 0, :], -1.0, None,
                                        ALU.mult)
                nc.vector.tensor_copy(krows[0:1, :], ks_r[:])
                nc.sync.dma_start(krows[1:3, :], kmsb[0:1, :, :])
                Xp = pC.tile([64, N2], f32, name="Xp", tag="tb")
                nc.tensor.matmul(Xp[:], lhsT=grid3, rhs=krows[:], start=True,
                                 stop=True)
                nc.scalar.activation(Esb[:], Xp[:], AF.Exp)
                vrp = pC.tile([64, N2], f32, name="vrp", tag="tb")
                nc.tensor.matmul(vrp[:], lhsT=ones_row[0:1, 0:64], rhs=vs_r[:],
                                 start=True, stop=True)
                nc.vector.tensor_tensor(Evm[:], Esb[:], vrp[:], ALU.mult)
                nc.vector.reduce_sum(numv[:], Evm[:], axis=AX.X)
                nc.vector.reduce_sum(denv[:], Esb[:], axis=AX.X)
                nc.vector.reciprocal_approx_fast(dinv[:], denv[:])
                nc.vector.tensor_tensor(Tcol[:], numv[:], dinv[:], ALU.mult)
                nc.vector.memset(Ts1[:], 0.0)
                nc.vector.memset(mcol[:], 0.0)
                nc.vector.memset(msh[:], 0.0)
                nc.scalar.dma_start(Ts1[0:62, :], Tcol[1:63, :])
                nc.vector.tensor_tensor(mcol[0:62, :], Ts1[0:62, :],
                                        Tcol[0:62, :], ALU.subtract)
                nc.sync.dma_start(msh[1:63, :], mcol[0:62, :])
                nc.vector.tensor_tensor(ccol[:], mcol[:], msh[:], ALU.subtract)
                nc.vector.tensor_scalar(msrow[:], msum_r, Tcol[0:1, 0:1], None,
                                        ALU.mult)
                nc.vector.tensor_tensor(browf[:], msrow[:], bpr_r, ALU.add)
                nc.vector.tensor_copy(brow16[:], browf[:])
                nc.sync.dma_start(mm16[8:9, :], brow16[:])
                nc.scalar.dma_start(mm16[72:73, :], brow16[:])

                # d + relu + gather; evens feed groups 0/1, odds 2/3;
                # y for groups 0/1 overlaps the odd half
                wp = pW.tile([128, 1024], f32)
                dp_q = {}

                def dmm(cc):
                    sp = 32 * (cc % 4)
                    dp = pD.tile([128, 1024], f32, name=f"dp{cc}", tag="dp")
                    for q in range(2):
                        for cq in range(2):
                            base = 2048 * cc + 1024 * q + 512 * cq
                            nc.tensor.matmul(
                                dp[64 * q:64 * q + 64, 512 * cq:512 * (cq + 1)],
                                lhsT=onesd[sp:sp + 1, 0:64],
                                rhs=trhs[sp:sp + 1, base:base + 512],
                                start=True, stop=True,
                                tile_position=(sp, 64 * q),
                            )
                    dp_q[cc] = dp

                def relu(cc):
                    dp = dp_q.pop(cc)
                    if cc % 2 == 0:
                        nc.scalar.activation(bas[:, cc, :], dp[:], AF.Relu,
                                             bias=miota)
                    else:
                        nc.vector.tensor_scalar(bas[:, cc, :], dp[:], miota, 0.0,
                                                ALU.add, ALU.max)

                def gmm(a, h):
                    nh, q = divmod(a, 2)
                    cc = 2 * h + nh
                    for cq in range(2):
                        nc.tensor.matmul(
                            wp[32 * a:32 * a + 8, 512 * cq:512 * (cq + 1)],
                            lhsT=crepv[64 * q:64 * q + 64, 8 * h:8 * h + 8],
                            rhs=bas[64 * q:64 * q + 64, cc,
                                    512 * cq:512 * (cq + 1)],
                            start=(h == 0), stop=(h == 7),
                            tile_position=(64 * q, 32 * a),
                            skip_group_check=True,
                        )

                with (
                    tc.tile_pool(name="ysq", bufs=6) as ysq,
                    tc.tile_pool(name="wgq", bufs=4) as wgq,
                ):
                    wgs = {}

                    def evac(a):
                        wg = wgq.tile([73, 1024], f16, name=f"wg{a}", tag="wg")
                        nc.vector.tensor_copy(wg[0:8, :], wp[32 * a:32 * a + 8, :])
                        nc.scalar.copy(wg[64:72, :], wp[32 * a:32 * a + 8, :])
                        nc.gpsimd.dma_start(wg[8:9, :], ones1k[:])
                        nc.sync.dma_start(wg[72:73, :], ones1k[:])
                        wgs[a] = wg

                    def ymm(a, j):
                        nb = 8 * a + j
                        sp = 64 * ((a + j) % 2)
                        pool, tg = ((pC, "tb") if nb % 2 == 0 else (pD, "dp"))
                        yp = pool.tile([128, C], f32, name=f"yp{nb}", tag=tg)
                        nc.tensor.matmul(
                            yp[:], lhsT=wgs[a][sp:sp + 9, 128 * j:128 * (j + 1)],
                            rhs=mm16[sp:sp + 9, :], start=True, stop=True,
                            tile_position=(sp, 0))
                        ysb = ysq.tile([128, C], f16, name=f"ysb{nb}", tag="ysb")
                        if nb % 2 == 0:
                            nc.vector.tensor_copy(ysb[:], yp[:])
                        else:
                            nc.scalar.copy(ysb[:], yp[:])
                        ydma = (nc.gpsimd.dma_start, nc.sync.dma_start,
                                nc.scalar.dma_start)[nb % 3]
                        ydma(y_d[128 * nb:128 * (nb + 1), :], ysb[:])

                    for cc in range(NCH):
                        dmm(cc)
                        relu(cc)
                    for h in range(8):
                        for a in range(4):
                            gmm(a, h)
                    for a in range(4):
                        evac(a)
                    for a in range(4):
                        for j in range(8):
                            ymm(a, j)

                    if debug:
                        nc.sync.dma_start(
                            dbg_t[:],
                            trhs[0:1, :].rearrange("p (h n) -> p h n", h=8))
                        nc.sync.dma_start(dbg_T[:, 0:1], Tcol[:])
                        nc.sync.dma_start(dbg_T[:, 1:2], ccol[:])
                        nc.sync.dma_start(dbg_T[:, 2:3], mcol[:])
                        nc.sync.dma_start(dbg_T[:, 3:4], Ts1[:])
                        nc.sync.dma_start(dbg_kv[:, 0:N2], ks_r[:])
                        nc.sync.dma_start(dbg_kv[:, N2:2 * N2], vs_r[:])
                        nc.sync.dma_start(dbg_xn[:, 0:N2], xn[:, 0, :])
                        nc.sync.dma_start(dbg_xn[:, N2:2 * N2], xn[:, 1, :])

    nc.compile()
    return nc


def _host_precompute(Wq, Wkv, Wsr, bsr, gamma, beta, Wproj, bproj, k_learn, v_learn):
    lksum = k_learn.reshape(HEADS, HC).sum(1)
    wqs = (Wq.reshape(C, HEADS, HC).sum(2) * (SCALE * lksum)[None, :]).astype(np.float32)
    wk = Wkv[:, 0::2].sum(1)
    wv = Wkv[:, 1::2].sum(1)
    wkv2 = np.stack([gamma * wk, gamma * wv], 1).astype(np.float32)
    lv = v_learn.reshape(HEADS, HC)
    Mmat = np.zeros((HEADS, C), np.float32)
    for h in range(HEADS):
        Mmat[h] = lv[h] @ Wproj[h::HEADS]
    Wsr_flat = np.ascontiguousarray(
        Wsr.transpose(2, 3, 1, 0).reshape(SR * SR * C, C)
    ).astype(np.float16)

    larr = np.arange(1, L + 1).astype(np.float64)
    larr[L - 1] = float(L - 1)
    g = BW * np.arctanh((larr - L / 2) / AMP)

    b32a = np.zeros((128, 8), np.float32)
    b32a[:, 0] = float(L // 2) - ((np.arange(128) % 64) + 1.0)
    b32a[:, 1:3] = bsr.reshape(2, 128).T
    b32a[:, 3:7] = wkv2.reshape(2, 128, 2).transpose(1, 0, 2).reshape(128, 4)

    b32b = np.zeros((4, 648), np.float32)
    b32b[0, 0:64] = g
    b32b[1, 0:64] = np.maximum(g, 0)
    b32b[2, 0:64] = np.maximum(-g, 0)
    e8 = np.zeros((8, 8), np.float32)
    np.fill_diagonal(e8, 1.0)
    b32b[0, 64:128] = e8.reshape(-1)
    b32b[0, 128:384] = Mmat.sum(0)
    b32b[0, 384:640] = bproj
    b32b[0, 640] = float((beta * wk).sum())
    b32b[0, 641] = float((beta * wv).sum())

    wq16 = wqs.reshape(2, 128, HEADS).transpose(1, 0, 2).reshape(128, 16).astype(np.float16)
    mm8 = Mmat.astype(np.float16)
    return dict(wsr=Wsr_flat, b32a=b32a, b32b=b32b, wq16=wq16, mm8=mm8)


def kernel(**inputs):
    x = np.asarray(inputs["x"], np.float32)
    weights = _host_precompute(
        *[np.asarray(inputs[k], np.float32) for k in
          ("Wq", "Wkv", "Wsr", "bsr", "gamma", "beta", "Wproj", "bproj",
           "k_learn", "v_learn")]
    )
    if "nc" not in _NC_CACHE:
        _NC_CACHE["nc"] = _build_nc()
    nc = _NC_CACHE["nc"]
    in_maps = [
        {"xt": np.ascontiguousarray(x[i].T).astype(np.float16), **weights}
        for i in range(B)
    ]
    from concourse.bass_utils import run_bass_kernel_spmd

    res = run_bass_kernel_spmd(nc, in_maps, core_ids=list(range(B)))
    y = np.stack([res.results[i]["y"].astype(np.float32) for i in range(B)], 0)
    return y


# revision 6
# speedup vs baseline: 1.1938x; 1.0708x over previous
"""Trainium2 Bass kernel for nn_Attention_17489106830121 (v3: L=64 stacked).

Same math as v2 (rank-1 logits -> per-core scalar function f(s) evaluated
by exact piecewise-linear interpolation on a warped grid), with:
  - L=64 knots (sim rel err 5.5e-3 vs 2e-2 tolerance), t = 32+31.3*tanh(s/3)
  - stacked layout: each [128, 1024] PSUM chunk holds TWO 64-row l-halves
    covering 2048 flat (h,n) columns -> relu pass halves (per-column cost)
  - 16 chunks; d = rank-1 broadcast matmuls (2 per chunk, col positions
    0/64, row strips 0/32/64/96); relu folds the knot offset via per-
    partition bias (ScalarE) / fused add,max (DVE), alternating engines
  - gather: 4 groups = (n-half nh, l-half q) at tile_position (64q, 32a),
    8 accumulating matmuls each (variant-8 lhsT, c placed at col h),
    interleaved with the d stream in the PE queue
  - blobs shrunk to ~22KB (b32a [128,8], b32b [4,640], wqs, mm8)
  - y per group right after its evac DMA
"""

import numpy as np

B, N, C, HEADS, SR = 8, 4096, 256, 8, 4
HC = C // HEADS
SCALE = HC ** -0.5
EPS = 1e-5
HS = 64 // SR
N2 = HS * HS             # 256
L = 64
BW = 3.0
AMP = 31.25              # exact in fp16; t = 32 + 31.25*tanh(s/3)
NCH = 16                 # [128,1024] stacked chunks; 2048 flat cols each

_NC_CACHE = {}


def _build_nc(debug=False):
    import concourse.bass as bass
    import concourse.bacc as bacc
    import concourse.mybir as mybir
    from concourse import tile

    dt = mybir.dt
    f32, f16 = dt.float32, dt.float16
    AF = mybir.ActivationFunctionType
    ALU = mybir.AluOpType
    AX = mybir.AxisListType

    nc = bacc.Bacc(None, target_bir_lowering=False)

    xt_d = nc.dram_tensor("xt", [2 * 128, N], f16, kind="ExternalInput")
    ws_d = nc.dram_tensor("wsr", [SR * SR * C, C], f16, kind="ExternalInput")
    b32a_d = nc.dram_tensor("b32a", [128, 8], f32, kind="ExternalInput")
    b32b_d = nc.dram_tensor("b32b", [4, 648], f32, kind="ExternalInput")
    wq_d = nc.dram_tensor("wq16", [128, 16], f16, kind="ExternalInput")
    mm_d = nc.dram_tensor("mm8", [8, C], f16, kind="ExternalInput")
    y_d = nc.dram_tensor("y", [N, C], f16, kind="ExternalOutput")
    if debug:
        dbg_t = nc.dram_tensor("dbg_t", [8, N], f16, kind="ExternalOutput")
        dbg_T = nc.dram_tensor("dbg_T", [64, 4], f32, kind="ExternalOutput")
        dbg_kv = nc.dram_tensor("dbg_kv", [1, 2 * N2], f32, kind="ExternalOutput")
        dbg_xn = nc.dram_tensor("dbg_xn", [128, 2 * N2], f32, kind="ExternalOutput")

    with tile.TileContext(nc) as tc:
        with tc.tile_pool(name="const", bufs=1) as cp:
            b32a = cp.tile([128, 8], f32)
            b32b = cp.tile([4, 648], f32)
            xt = cp.tile([128, 2, N], f16)
            wssb = cp.tile([128, 32, C], f16)
            trhs = cp.tile([97, 8 * N], f16)
            onesd = cp.tile([97, 128], f16)
            crepv = cp.tile([128, 64], f16)
            mm16 = cp.tile([73, C], f16)
            ones1k = cp.tile([1, 1024], f16)
            browf = cp.tile([1, C], f32)
            brow16 = cp.tile([1, C], f16)
            wqssb = cp.tile([128, 2, HEADS], f16)
            murep = cp.tile([128, N2], f32)
            rsrep = cp.tile([128, N2], f32)
            ones_row = cp.tile([1, 128], f32)
            ones_col = cp.tile([128, 1], f32)
            mones256 = cp.tile([1, N2], f32)
            eps_sb = cp.tile([128, 1], f32)
            cvo = cp.tile([128, 2, N2], f32)
            xm = cp.tile([128, 2, N2], f32)
            xn = cp.tile([128, 2, N2], f32)
            sq2 = cp.tile([128, 2, N2], f32)
            murow = cp.tile([1, 2, N2], f32)
            varrow = cp.tile([1, N2], f32)
            lvrow = cp.tile([1, N2], f32)
            rstdrow = cp.tile([1, N2], f32)
            ks_r = cp.tile([1, N2], f32)
            vs_r = cp.tile([1, N2], f32)
            kmx = cp.tile([1, 1], f32)
            kmn = cp.tile([1, 1], f32)
            kmsb = cp.tile([1, 2, N2], f32)
            krows = cp.tile([3, N2], f32)
            Esb = cp.tile([64, N2], f32)
            Evm = cp.tile([64, N2], f32)
            numv = cp.tile([64, 1], f32)
            denv = cp.tile([64, 1], f32)
            dinv = cp.tile([64, 1], f32)
            Tcol = cp.tile([64, 1], f32)
            Ts1 = cp.tile([64, 1], f32)
            mcol = cp.tile([64, 1], f32)
            msh = cp.tile([64, 1], f32)
            ccol = cp.tile([64, 1], f32)
            cTrow = cp.tile([1, 64], f32)
            msrow = cp.tile([1, C], f32)

            miota = b32a[:, 0:1]
            bsrcol = b32a[:, 1:3]
            wkvsb = b32a[:, 3:7].rearrange("p (t h) -> p t h", t=2)
            grid3 = b32b[0:3, 0:64]
            E8row = b32b[0:1, 64:128]
            msum_r = b32b[0:1, 128:384]
            bpr_r = b32b[0:1, 384:640]
            cb_r = b32b[0:1, 640:642]

            # ----- input DMAs: xt 6 pieces across all 3 queues, then wssb
            # staged in conv consumption order -----
            xd = xt_d[:].rearrange("(ct p) n -> p ct n", p=128)
            wsv = ws_d[:].rearrange("(t p) c -> p t c", p=128)
            nc.sync.dma_start(xt[:, 0, 0:2731], xd[:, 0, 0:2731])
            nc.scalar.dma_start(xt[:, 1, 0:2731], xd[:, 1, 0:2731])
            nc.gpsimd.dma_start(xt[:, 0, 2731:4096], xd[:, 0, 2731:4096])
            nc.gpsimd.dma_start(xt[:, 1, 2731:4096], xd[:, 1, 2731:4096])
            nc.gpsimd.dma_start(wqssb[:], wq_d[:].rearrange("p (t h) -> p t h", t=2))
            nc.gpsimd.dma_start(b32a[:], b32a_d[:])
            nc.gpsimd.dma_start(wssb[:, 0:8, :], wsv[:, 0:8, :])
            nc.sync.dma_start(wssb[:, 8:16, :], wsv[:, 8:16, :])
            nc.scalar.dma_start(wssb[:, 16:24, :], wsv[:, 16:24, :])
            nc.sync.dma_start(wssb[:, 24:28, :], wsv[:, 24:28, :])
            nc.scalar.dma_start(wssb[:, 28:32, :], wsv[:, 28:32, :])
            nc.gpsimd.dma_start(b32b[:], b32b_d[:])
            nc.gpsimd.dma_start(mm16[0:8, :], mm_d[:])
            nc.gpsimd.dma_start(mm16[64:72, :], mm_d[:])
            nc.vector.memset(ones_row[:], 1.0)
            nc.vector.memset(mones256[:], -1.0)
            nc.vector.memset(eps_sb[:], EPS)
            nc.vector.memset(ones_col[:], 1.0)
            nc.vector.memset(ones1k[:], 1.0)
            for sp in (0, 32, 64, 96):
                nc.vector.memset(onesd[sp:sp + 1, :], AMP)

            # ----- s^T, warp to t, flatten; conv/LN shares the PSUM scope --
            with (
                tc.tile_pool(name="ssp", bufs=1) as ssp,
                tc.tile_pool(name="psSB", bufs=2, space="PSUM") as pB,
            ):
                t16 = ssp.tile([8, N], f16)
                if True:
                    pS = pB
                    for k4 in range(2):
                        sps = pS.tile([128, 512], f32, name="sps", tag="sps")
                        for q in range(4):
                            k = 4 * k4 + q
                            cb = 32 * q
                            for ct in range(2):
                                nc.tensor.matmul(
                                    sps[cb:cb + 8, :],
                                    lhsT=wqssb[:, ct, :],
                                    rhs=xt[:, ct, 512 * k:512 * (k + 1)],
                                    start=(ct == 0), stop=(ct == 1),
                                    tile_position=(0, cb),
                                )
                        for q in range(4):
                            k = 4 * k4 + q
                            sl = slice(512 * k, 512 * (k + 1))
                            nc.scalar.activation(t16[:, sl],
                                                 sps[32 * q:32 * q + 8, :],
                                                 AF.Tanh, scale=1.0 / BW)
                    for k in range(8):
                        sl = slice(512 * k, 512 * (k + 1))
                        for i, sp in enumerate((0, 32, 64, 96)):
                            eng2 = (nc.sync, nc.scalar, nc.gpsimd,
                                    nc.sync)[(4 * k + i) % 4]
                            eng2.dma_start(
                                trhs[sp:sp + 1, :].rearrange(
                                    "p (h n) -> p h n", h=8)[:, :, sl],
                                t16[:, sl],
                            )
                # ----- conv (out [c_out, spatial]) + LN stats -----
                xtr = xt[:].rearrange("p ct (ph kh pw kw) -> p ct kh kw ph pw",
                                      ph=16, kh=4, pw=16, kw=4)
                for mo in range(2):
                    cpsA = pB.tile([128, N2], f32, name=f"cpsA{mo}", tag="cpsA",
                                   bufs=1)
                    cpsB = pB.tile([128, N2], f32, name=f"cpsB{mo}", tag="cpsB",
                                   bufs=1)
                    for kh in range(4):
                        for kw in range(4):
                            for ct in range(2):
                                kidx = kh * 8 + kw * 2 + ct
                                cnt = kh * 8 + kw * 2 + ct
                                nc.tensor.matmul(
                                    cpsA[:],
                                    lhsT=wssb[0:64, kidx, 128 * mo:128 * (mo + 1)],
                                    rhs=xtr[0:64, ct, kh, kw],
                                    start=(cnt == 0), stop=(cnt == 31),
                                    tile_position=(0, 0),
                                )
                                nc.tensor.matmul(
                                    cpsB[:],
                                    lhsT=wssb[64:128, kidx, 128 * mo:128 * (mo + 1)],
                                    rhs=xtr[64:128, ct, kh, kw],
                                    start=(cnt == 0), stop=(cnt == 31),
                                    tile_position=(64, 0),
                                )
                    nc.vector.tensor_scalar(cvo[:, mo, :], cpsA[:],
                                            bsrcol[:, mo:mo + 1], None, ALU.add)
                    nc.vector.tensor_tensor(cvo[:, mo, :], cvo[:, mo, :],
                                            cpsB[:], ALU.add)
                nc.vector.tensor_tensor(sq2[:], cvo[:], cvo[:], ALU.mult)
                muA = pB.tile([1, N2], f32, name="muA", tag="muA", bufs=1)
                muB = pB.tile([1, N2], f32, name="muB", tag="muB", bufs=1)
                for mo in range(2):
                    nc.tensor.matmul(muA[:], lhsT=ones_col[:],
                                     rhs=cvo[:, mo, :], start=(mo == 0),
                                     stop=(mo == 1))
                    nc.tensor.matmul(muB[:], lhsT=ones_col[:],
                                     rhs=sq2[:, mo, :], start=(mo == 0),
                                     stop=(mo == 1))
                nc.vector.tensor_scalar(murow[:, 0, :], muA[:], 1.0 / N2, None,
                                        ALU.mult)
                nc.vector.tensor_scalar(murow[:, 1, :], muB[:], 1.0 / N2, None,
                                        ALU.mult)
                nc.vector.tensor_tensor(varrow[:], murow[:, 0, :], murow[:, 0, :],
                                        ALU.mult)
                nc.vector.tensor_tensor(varrow[:], murow[:, 1, :], varrow[:],
                                        ALU.subtract)
                nc.scalar.activation(lvrow[:], varrow[:], AF.Ln,
                                     bias=eps_sb[0:1, :])
                nc.scalar.activation(rstdrow[:], lvrow[:], AF.Exp, scale=-0.5)
                mrp = pB.tile([128, 2, N2], f32, name="mrp", tag="mrp", bufs=1)
                nc.tensor.matmul(mrp[:, 0, :], lhsT=ones_row[:], rhs=murow[:, 0, :],
                                 start=True, stop=True)
                nc.tensor.matmul(mrp[:, 1, :], lhsT=ones_row[:], rhs=rstdrow[:],
                                 start=True, stop=True)
                nc.vector.tensor_copy(murep[:], mrp[:, 0, :])
                nc.vector.tensor_copy(rsrep[:], mrp[:, 1, :])
                for mo in range(2):
                    nc.vector.tensor_tensor(xm[:, mo, :], cvo[:, mo, :], murep[:],
                                            ALU.subtract)
                    nc.vector.tensor_tensor(xn[:, mo, :], xm[:, mo, :], rsrep[:],
                                            ALU.mult)

            # ----- middle: table + d/relu/gather interleaved + y -----
            with (
                tc.tile_pool(name="mid", bufs=1) as mp,
                tc.tile_pool(name="pD", bufs=2, space="PSUM") as pD,
                tc.tile_pool(name="pW", bufs=1, space="PSUM") as pW,
                tc.tile_pool(name="pC", bufs=2, space="PSUM") as pC,
            ):
                bas = mp.tile([128, NCH, 1024], f16)

                # table build
                kps_k = pC.tile([1, N2], f32, name="kps_k", tag="tb")
                kps_v = pC.tile([1, N2], f32, name="kps_v", tag="tb")
                for mo in range(2):
                    nc.tensor.matmul(kps_k[:], lhsT=wkvsb[:, mo, 0:1],
                                     rhs=xn[:, mo, :], start=(mo == 0), stop=(mo == 1))
                    nc.tensor.matmul(kps_v[:], lhsT=wkvsb[:, mo, 1:2],
                                     rhs=xn[:, mo, :], start=(mo == 0), stop=(mo == 1))
                nc.vector.tensor_scalar(ks_r[:], kps_k[:], cb_r[0:1, 0:1], None,
                                        ALU.add)
                nc.vector.tensor_scalar(vs_r[:], kps_v[:], cb_r[0:1, 1:2], None,
                                        ALU.add)
                nc.vector.reduce_max(kmx[:], ks_r[:], axis=AX.X)
                nc.vector.tensor_reduce(kmn[:], ks_r[:], axis=AX.X, op=ALU.min)
                kmp = pC.tile([1, 2, N2], f32, name="kmp", tag="tb")
                nc.tensor.matmul(kmp[:, 0, :], lhsT=kmx[:], rhs=mones256[:],
                                 start=True, stop=True)
                nc.tensor.matmul(kmp[:, 1, :], lhsT=kmn[:], rhs=mones256[:],
                                 start=True, stop=True)
                nc.vector.tensor_scalar(kmsb[:], kmp[:], -1.0, None, ALU.mult)
                nc.vector.tensor_scalar(kmsb[:, 0, :], kmsb[:, 0, :], -1.0, None,
                                        ALU.mult)
                nc.vector.tensor_copy(krows[0:1, :], ks_r[:])
                nc.sync.dma_start(krows[1:3, :], kmsb[0:1, :, :])
                Xp = pC.tile([64, N2], f32, name="Xp", tag="tb")
                nc.tensor.matmul(Xp[:], lhsT=grid3, rhs=krows[:], start=True,
                                 stop=True)
                nc.scalar.activation(Esb[:], Xp[:], AF.Exp)
                vrp = pC.tile([64, N2], f32, name="vrp", tag="tb")
                nc.tensor.matmul(vrp[:], lhsT=ones_row[0:1, 0:64], rhs=vs_r[:],
                                 start=True, stop=True)
                nc.vector.tensor_tensor(Evm[:], Esb[:], vrp[:], ALU.mult)
                nc.vector.reduce_sum(numv[:], Evm[:], axis=AX.X)
                nc.vector.reduce_sum(denv[:], Esb[:], axis=AX.X)
                nc.vector.reciprocal_approx_fast(dinv[:], denv[:])
                nc.vector.tensor_tensor(Tcol[:], numv[:], dinv[:], ALU.mult)
                nc.vector.memset(Ts1[:], 0.0)
                nc.vector.memset(mcol[:], 0.0)
                nc.vector.memset(msh[:], 0.0)
                nc.scalar.dma_start(Ts1[0:62, :], Tcol[1:63, :])
                nc.vector.tensor_tensor(mcol[0:62, :], Ts1[0:62, :],
                                        Tcol[0:62, :], ALU.subtract)
                nc.sync.dma_start(msh[1:63, :], mcol[0:62, :])
                nc.vector.tensor_tensor(ccol[:], mcol[:], msh[:], ALU.subtract)
                nc.vector.tensor_scalar(msrow[:], msum_r, Tcol[0:1, 0:1], None,
                                        ALU.mult)
                nc.vector.tensor_tensor(browf[:], msrow[:], bpr_r, ALU.add)
                nc.vector.tensor_copy(brow16[:], browf[:])
                nc.sync.dma_start(mm16[8:9, :], brow16[:])
                nc.scalar.dma_start(mm16[72:73, :], brow16[:])

                # d + relu + gather; evens feed groups 0/1, odds 2/3;
                # y for groups 0/1 overlaps the odd half
                wp = pW.tile([128, 1024], f32)
                dp_q = {}

                def dmm(cc):
                    sp = 32 * (cc % 4)
                    dp = pD.tile([128, 1024], f32, name=f"dp{cc}", tag="dp")
                    for q in range(2):
                        for cq in range(2):
                            base = 2048 * cc + 1024 * q + 512 * cq
                            nc.tensor.matmul(
                                dp[64 * q:64 * q + 64, 512 * cq:512 * (cq + 1)],
                                lhsT=onesd[sp:sp + 1, 0:64],
                                rhs=trhs[sp:sp + 1, base:base + 512],
                                start=True, stop=True,
                                tile_position=(sp, 64 * q),
                            )
                    dp_q[cc] = dp

                def relu(cc):
                    dp = dp_q.pop(cc)
                    if cc % 3 != 1:
                        nc.scalar.activation(bas[:, cc, :], dp[:], AF.Relu,
                                             bias=miota)
                    else:
                        nc.vector.tensor_scalar(bas[:, cc, :], dp[:], miota, 0.0,
                                                ALU.add, ALU.max)

                def gmm(a, h):
                    nh, q = divmod(a, 2)
                    cc = 2 * h + nh
                    for cq in range(2):
                        nc.tensor.matmul(
                            wp[32 * a:32 * a + 8, 512 * cq:512 * (cq + 1)],
                            lhsT=crepv[64 * q:64 * q + 64, 8 * h:8 * h + 8],
                            rhs=bas[64 * q:64 * q + 64, cc,
                                    512 * cq:512 * (cq + 1)],
                            start=(h == 0), stop=(h == 7),
                            tile_position=(64 * q, 32 * a),
                            skip_group_check=True,
                        )

                with (
                    tc.tile_pool(name="ysq", bufs=6) as ysq,
                    tc.tile_pool(name="wgq", bufs=4) as wgq,
                ):
                    wgs = {}

                    def evac(a):
                        wg = wgq.tile([73, 1024], f16, name=f"wg{a}", tag="wg")
                        nc.vector.tensor_copy(wg[0:8, :], wp[32 * a:32 * a + 8, :])
                        nc.scalar.copy(wg[64:72, :], wp[32 * a:32 * a + 8, :])
                        nc.gpsimd.dma_start(wg[8:9, :], ones1k[:])
                        nc.sync.dma_start(wg[72:73, :], ones1k[:])
                        wgs[a] = wg

                    def ymm(a, j):
                        nb = 8 * a + j
                        sp = 64 * ((a + j) % 2)
                        pool, tg = ((pC, "tb") if nb % 2 == 0 else (pD, "dp"))
                        yp = pool.tile([128, C], f32, name=f"yp{nb}", tag=tg)
                        nc.tensor.matmul(
                            yp[:], lhsT=wgs[a][sp:sp + 9, 128 * j:128 * (j + 1)],
                            rhs=mm16[sp:sp + 9, :], start=True, stop=True,
                            tile_position=(sp, 0))
                        ysb = ysq.tile([128, C], f16, name=f"ysb{nb}", tag="ysb")
                        if nb % 2 == 0:
                            nc.vector.tensor_copy(ysb[:], yp[:])
                        else:
                            nc.scalar.copy(ysb[:], yp[:])
                        ydma = (nc.gpsimd.dma_start, nc.sync.dma_start,
                                nc.scalar.dma_start)[nb % 3]
                        ydma(y_d[128 * nb:128 * (nb + 1), :], ysb[:])

                    for cc in range(NCH):
                        dmm(cc)
                        relu(cc)
                    for h in range(8):
                        for a in range(4):
                            gmm(a, h)
                    for a in range(4):
                        evac(a)
                    for a in range(4):
                        for j in range(8):
                            ymm(a, j)

                    if debug:
                        nc.sync.dma_start(
                            dbg_t[:],
                            trhs[0:1, :].rearrange("p (h n) -> p h n", h=8))
                        nc.sync.dma_start(dbg_T[:, 0:1], Tcol[:])
                        nc.sync.dma_start(dbg_T[:, 1:2], ccol[:])
                        nc.sync.dma_start(dbg_T[:, 2:3], mcol[:])
                        nc.sync.dma_start(dbg_T[:, 3:4], Ts1[:])
                        nc.sync.dma_start(dbg_kv[:, 0:N2], ks_r[:])
                        nc.sync.dma_start(dbg_kv[:, N2:2 * N2], vs_r[:])
                        nc.sync.dma_start(dbg_xn[:, 0:N2], xn[:, 0, :])
                        nc.sync.dma_start(dbg_xn[:, N2:2 * N2], xn[:, 1, :])

    nc.compile()
    return nc


def _host_precompute(Wq, Wkv, Wsr, bsr, gamma, beta, Wproj, bproj, k_learn, v_learn):
    lksum = k_learn.reshape(HEADS, HC).sum(1)
    wqs = (Wq.reshape(C, HEADS, HC).sum(2) * (SCALE * lksum)[None, :]).astype(np.float32)
    wk = Wkv[:, 0::2].sum(1)
    wv = Wkv[:, 1::2].sum(1)
    wkv2 = np.stack([gamma * wk, gamma * wv], 1).astype(np.float32)
    lv = v_learn.reshape(HEADS, HC)
    Mmat = np.zeros((HEADS, C), np.float32)
    for h in range(HEADS):
        Mmat[h] = lv[h] @ Wproj[h::HEADS]
    Wsr_flat = np.ascontiguousarray(
        Wsr.transpose(2, 3, 1, 0).reshape(SR * SR * C, C)
    ).astype(np.float16)

    larr = np.arange(1, L + 1).astype(np.float64)
    larr[L - 1] = float(L - 1)
    g = BW * np.arctanh((larr - L / 2) / AMP)

    b32a = np.zeros((128, 8), np.float32)
    b32a[:, 0] = float(L // 2) - ((np.arange(128) % 64) + 1.0)
    b32a[:, 1:3] = bsr.reshape(2, 128).T
    b32a[:, 3:7] = wkv2.reshape(2, 128, 2).transpose(1, 0, 2).reshape(128, 4)

    b32b = np.zeros((4, 648), np.float32)
    b32b[0, 0:64] = g
    b32b[1, 0:64] = np.maximum(g, 0)
    b32b[2, 0:64] = np.maximum(-g, 0)
    e8 = np.zeros((8, 8), np.float32)
    np.fill_diagonal(e8, 1.0)
    b32b[0, 64:128] = e8.reshape(-1)
    b32b[0, 128:384] = Mmat.sum(0)
    b32b[0, 384:640] = bproj
    b32b[0, 640] = float((beta * wk).sum())
    b32b[0, 641] = float((beta * wv).sum())

    wq16 = wqs.reshape(2, 128, HEADS).transpose(1, 0, 2).reshape(128, 16).astype(np.float16)
    mm8 = Mmat.astype(np.float16)
    return dict(wsr=Wsr_flat, b32a=b32a, b32b=b32b, wq16=wq16, mm8=mm8)


def kernel(**inputs):
    x = np.asarray(inputs["x"], np.float32)
    weights = _host_precompute(
        *[np.asarray(inputs[k], np.float32) for k in
          ("Wq", "Wkv", "Wsr", "bsr", "gamma", "beta", "Wproj", "bproj",
           "k_learn", "v_learn")]
    )
    if "nc" not in _NC_CACHE:
        _NC_CACHE["nc"] = _build_nc()
    nc = _NC_CACHE["nc"]
    in_maps = [
        {"xt": np.ascontiguousarray(x[i].T).astype(np.float16), **weights}
        for i in range(B)
    ]
    from concourse.bass_utils import run_bass_kernel_spmd

    res = run_bass_kernel_spmd(nc, in_maps, core_ids=list(range(B)))
    y = np.stack([res.results[i]["y"].astype(np.float32) for i in range(B)], 0)
    return y
